# revision 1
# baseline (speedup 1.0000x reference)
"""Autoformer encoder block on 8 TRN2 NeuronCores.

Sharding: data-parallel over batch (B=8 -> 1 batch per core), weights
replicated. No collectives.

Per-core math (S=1024, D=512, H=8, dp=64, K=25):
  trend = movavg(x)               # banded matmul, token-major
  seas  = x - trend               # token-major, then PE-transpose -> seas.T
  q.T/k.T = wq/wk.T @ seas.T      # feature-major
  v     = seas @ wv               # token-major (for AV lhsT + V-sums)
  The reference's rfft/irfft over the depth axis (n=2S) makes
  corr[b,h,s,t] == 0 for t >= dp, so attention reduces to 64 depth-lags:
    corr.T = IDFT @ (QF (*) conj(KF)), QF = FWD.T @ q.T   (n=128 DFT)
    E = exp(corr/8); out = (E @ v[:64] + (Vsum - Vhead)) / (rowsum(E)+S-dp)
  wo, LN1, FFN(4x, relu), LN2 feature-major (stats via ones-matmul).
  seasonal_out + trend == x_out exactly (trend2 cancels), so movavg2 is
  skipped; final LN3 runs token-major after a PE-transpose, then DMA out.
"""

import numpy as np

B, S, D, H = 8, 1024, 512, 8
DP = D // H
DH = 4 * D
KWIN, PAD = 25, 12
EPS = 1e-6
NCORES = 8
NT = S // 128   # 8 token tiles
ND = D // 128   # 4 feature tiles
NH = DH // 128  # 16 hidden tiles

_CACHE = {}


def _consts():
    c = {}
    # moving-average 3-piece band blocks over token-major x tiles:
    # trend tile j = BM[:,j]^T x[j] + BP[:,j]^T x[j-1][116:128] + BN[:,j]^T x[j+1][0:12]
    cnt = np.minimum(S, np.arange(S) + PAD + 1) - np.maximum(0, np.arange(S) - PAD)
    BM = np.zeros((128, NT * 128), np.float32)
    # BPN rows 0:12 = next-tile piece, rows 64:128 = prev-tile piece
    BPN = np.zeros((128, NT * 128), np.float32)
    for j in range(NT):
        for cc in range(128):
            s = 128 * j + cc
            for i in range(128):
                if abs(i - cc) <= PAD:
                    BM[i, 128 * j + cc] = 1.0 / cnt[s]
            for i in range(64, 128):
                if j >= 1 and abs(128 * (j - 1) + i - s) <= PAD:
                    BPN[i, 128 * j + cc] = 1.0 / cnt[s]
            for i in range(PAD):
                if j < NT - 1 and abs(128 * (j + 1) + i - s) <= PAD:
                    BPN[i, 128 * j + cc] = 1.0 / cnt[s]
    c["BM"], c["BPN"] = BM, BPN

    import ml_dtypes
    bf = ml_dtypes.bfloat16

    # packed forward DFT (n=128): FQ [64,128] = [cos f=0..64 | sin f=1..63],
    # FK2 [64,128] = [sin f=0..64 | cos f=1..63]; doubled over partitions so
    # heads at base 0 and base 64 can slice the same constant.
    n = 2 * DP
    d = np.arange(DP)[:, None]
    f65 = np.arange(65)[None, :]
    f63 = np.arange(1, 64)[None, :]
    FQ = np.concatenate([np.cos(2 * np.pi * f65 * d / n),
                         np.sin(2 * np.pi * f63 * d / n)], axis=1)
    FK2 = np.concatenate([np.sin(2 * np.pi * f65 * d / n),
                          np.cos(2 * np.pi * f63 * d / n)], axis=1)
    c["FQD"] = np.concatenate([FQ, FQ], axis=0).astype(bf)
    c["FK2D"] = np.concatenate([FK2, FK2], axis=0).astype(bf)

    # packed inverse (softmax scale 1/sqrt(dp) folded in):
    # corr'[t] = IA^T @ (qf*k1f) + IB^T @ (qf*k2f)
    t = np.arange(DP)[None, :]
    w = np.full(65, 2.0); w[0] = 1.0; w[64] = 1.0
    fc = np.arange(65)[:, None]
    fs = np.arange(1, 64)[:, None]
    scale = 1.0 / np.sqrt(DP)
    IA = np.concatenate([(w[:, None] / n) * np.cos(2 * np.pi * fc * t / n),
                         (2.0 / n) * np.cos(2 * np.pi * fs * t / n)], axis=0)
    IB = np.concatenate([-(w[:, None] / n) * np.sin(2 * np.pi * fc * t / n),
                         (2.0 / n) * np.sin(2 * np.pi * fs * t / n)], axis=0)
    c["IA"] = (scale * IA).astype(bf)
    c["IB"] = (scale * IB).astype(bf)

    # head-pair selector for 1/Z broadcast: pass p covers heads 2p, 2p+1
    HSEL = np.zeros((8, 4 * 128), np.float32)
    for p in range(4):
        for mm_ in range(128):
            HSEL[2 * p + mm_ // 64, p * 128 + mm_] = 1.0
    c["HSEL"] = HSEL.astype(bf)
    # Z rows: ZBLK[:, 8p+j] sums E-pair rows 0:64 into row 2p, 64:128 into 2p+1
    ZBLK = np.zeros((128, 32), np.float32)
    for p in range(4):
        ZBLK[0:64, 8 * p + 2 * p] = 1.0
        ZBLK[64:128, 8 * p + 2 * p + 1] = 1.0
    c["ZBLK"] = ZBLK.astype(bf)

    c["ones"] = np.ones((128, 128), np.float32)
    c["ident"] = np.eye(128, dtype=np.float32)
    return c


def _build():
    import concourse.bacc as bacc
    import concourse.mybir as mybir
    import concourse.tile as tile

    f32 = mybir.dt.float32
    f32r = mybir.dt.float32r
    Alu = mybir.AluOpType
    Act = mybir.ActivationFunctionType

    nc = bacc.Bacc()

    # ---- DRAM parameters (same names as setup_inputs keys) ----
    x_d = nc.declare_dram_parameter("x", [S, D], f32, isOutput=False)
    wq_d = nc.declare_dram_parameter("wq", [D, D], f32, isOutput=False)
    bq_d = nc.declare_dram_parameter("bq", [D], f32, isOutput=False)
    wk_d = nc.declare_dram_parameter("wk", [D, D], f32, isOutput=False)
    bk_d = nc.declare_dram_parameter("bk", [D], f32, isOutput=False)
    wv_d = nc.declare_dram_parameter("wv", [D, D], f32, isOutput=False)
    wo_d = nc.declare_dram_parameter("wo", [D, D], f32, isOutput=False)
    bo_d = nc.declare_dram_parameter("bo", [D], f32, isOutput=False)
    w1_d = nc.declare_dram_parameter("w1", [D, DH], f32, isOutput=False)
    b1_d = nc.declare_dram_parameter("b1", [DH], f32, isOutput=False)
    w2_d = nc.declare_dram_parameter("w2", [DH, D], f32, isOutput=False)
    b2_d = nc.declare_dram_parameter("b2", [D], f32, isOutput=False)
    g1_d = nc.declare_dram_parameter("g1", [D], f32, isOutput=False)
    be1_d = nc.declare_dram_parameter("be1", [D], f32, isOutput=False)
    g2_d = nc.declare_dram_parameter("g2", [D], f32, isOutput=False)
    be2_d = nc.declare_dram_parameter("be2", [D], f32, isOutput=False)
    g3_d = nc.declare_dram_parameter("g3", [D], f32, isOutput=False)
    be3_d = nc.declare_dram_parameter("be3", [D], f32, isOutput=False)
    out_d = nc.declare_dram_parameter("out", [S, D], f32, isOutput=True)

    C = _consts()
    BM_d = nc.inline_tensor(C["BM"], name="c_bm")
    BPN_d = nc.inline_tensor(C["BPN"], name="c_bpn")
    FQD_d = nc.inline_tensor(C["FQD"], name="c_fqd")
    FK2D_d = nc.inline_tensor(C["FK2D"], name="c_fk2d")
    IA_d = nc.inline_tensor(C["IA"], name="c_ia")
    IB_d = nc.inline_tensor(C["IB"], name="c_ib")
    HSEL_d = nc.inline_tensor(C["HSEL"], name="c_hsel")
    ZBLK_d = nc.inline_tensor(C["ZBLK"], name="c_zblk")
    ones_d = nc.inline_tensor(C["ones"], name="c_ones")
    ident_d = nc.inline_tensor(C["ident"], name="c_ident")

    def r(ap):
        return ap.bitcast(f32r)

    def mm(out, lhsT, rhs, start=True, stop=True):
        nc.tensor.matmul(out, r(lhsT), r(rhs), start=start, stop=stop)

    with tile.TileContext(nc) as tc:
        with (
            tc.tile_pool(name="konst", bufs=1) as konst,
            tc.tile_pool(name="persist", bufs=1) as persist,
            tc.tile_pool(name="mid", bufs=1) as mid,
        ):
            # ---- consts to SBUF ----
            bfdt = mybir.dt.bfloat16
            fqd = konst.tile([128, 128], bfdt)
            fk2d = konst.tile([128, 128], bfdt)
            ia_b = konst.tile([128, DP], bfdt)
            ib_b = konst.tile([128, DP], bfdt)
            hsel_b = konst.tile([8, 4 * 128], bfdt)
            zblk = konst.tile([128, 32], bfdt)
            ones = konst.tile([128, 128], f32)
            ident = konst.tile([128, 128], f32)
            # small constant loads ride the scalar-engine trigger queue so
            # they don't serialize behind x/bands on the sync queue (each
            # DMA trigger costs ~0.65us of queue time)
            for tl, dr in ((fqd, FQD_d), (fk2d, FK2D_d), (ia_b, IA_d),
                           (ib_b, IB_d), (hsel_b, HSEL_d), (zblk, ZBLK_d)):
                nc.sync.dma_start(tl[:], dr[:])

            def col(dram, n):
                t = konst.tile([128, n], f32, tag="col_" + dram.name)
                nc.sync.dma_start(t[:], dram.rearrange("(j p) -> p j", p=128))
                return t
            bq_c, bk_c = col(bq_d, ND), col(bk_d, ND)
            bo_c, b2_c = col(bo_d, ND), col(b2_d, ND)
            b1_c = col(b1_d, NH)
            g1_c, be1_c = col(g1_d, ND), col(be1_d, ND)
            g2_c, be2_c = col(g2_d, ND), col(be2_d, ND)
            g3_r = konst.tile([1, D], f32)
            be3_r = konst.tile([1, D], f32)
            nc.sync.dma_start(r(g3_r[:]), r(g3_d[None, :]))
            nc.sync.dma_start(r(be3_r[:]), r(be3_d[None, :]))

            x_tok = persist.tile([128, NT * D], f32)
            seasT = persist.tile([128, ND * S], f32)
            wqkvo = []

            def wload(pool, dram, din, dout, eng=None):
                t = pool.tile([128, (din // 128) * dout], f32,
                              tag="w_" + dram.name)
                (eng or nc.sync).dma_start(
                    r(t[:].rearrange("p (k f) -> p k f", f=dout)),
                    r(dram.rearrange("(k p) f -> p k f", p=128)))
                return t

            # ============ phase 1: load x, moving average, transpose ========
            wop_p = tc.tile_pool(name="wop", bufs=1)
            wop = wop_p.__enter__()
            w1_ctx = tc.tile_pool(name="w1p", bufs=1)
            w1p = w1_ctx.__enter__()
            w2_ctx = tc.tile_pool(name="w2p", bufs=1)
            w2p = w2_ctx.__enter__()
            wat_ctx = tc.tile_pool(name="wat", bufs=1)
            wat = wat_ctx.__enter__()
            with (
                tc.tile_pool(name="ph1", bufs=1) as ph1,
                tc.tile_pool(name="ps1", bufs=2, space="PSUM") as ps1,
            ):
                bandm = ph1.tile([128, NT * 128], f32)
                bandpn = ph1.tile([128, NT * 128], f32)
                nc.sync.dma_start(r(bandm[:]), r(BM_d[:]))
                nc.sync.dma_start(r(bandpn[:]), r(BPN_d[:]))
                nc.sync.dma_start(r(ones[:]), r(ones_d[:]))
                nc.sync.dma_start(ident[:], ident_d[:])
                for g_ in range(2):
                    nc.sync.dma_start(
                        r(x_tok[:, g_ * 4 * D:(g_ + 1) * 4 * D].rearrange(
                            "p (st d) -> p st d", d=D)),
                        r(x_d[g_ * 512:(g_ + 1) * 512, :].rearrange(
                            "(st p) d -> p st d", p=128)))
                seas_tok = ph1.tile([128, NT * D], f32)
                for j in range(NT):
                    ps = ps1.tile([128, D], f32, tag="mavg")
                    mm(ps[:], bandm[:, 128 * j:128 * (j + 1)],
                       x_tok[:, j * D:(j + 1) * D], start=True, stop=False)
                    if j >= 1:
                        mm(ps[:], bandpn[64:128, 128 * j:128 * (j + 1)],
                           x_tok[64:128, (j - 1) * D:j * D],
                           start=False, stop=(j == NT - 1))
                    if j < NT - 1:
                        mm(ps[:], bandpn[0:PAD, 128 * j:128 * (j + 1)],
                           x_tok[0:PAD, (j + 1) * D:(j + 2) * D],
                           start=False, stop=True)
                    nc.vector.tensor_tensor(
                        seas_tok[:, j * D:(j + 1) * D],
                        x_tok[:, j * D:(j + 1) * D], ps[:], Alu.subtract)

                for st in range(NT):
                    for kt in range(ND):
                        ps = ps1.tile([128, 128], f32, tag="tr")
                        nc.tensor.transpose(
                            ps[:],
                            seas_tok[:, st * D + kt * 128:st * D + (kt + 1) * 128],
                            ident[:])
                        dst = r(seasT[:, kt * S + st * 128:kt * S + (st + 1) * 128])
                        if (st + kt) % 2 == 0:
                            nc.vector.tensor_copy(dst, ps[:])
                        else:
                            nc.scalar.copy(dst, ps[:])
                for dr_ in (wq_d, wk_d, wv_d):
                    wqkvo.append(wload(wat, dr_, D, D))
                wqkvo.append(wload(wop, wo_d, D, D))

            # ============ phase 2: attention ============
            out1T = mid.tile([128, ND * S], f32, tag="m16")
            with (
                tc.tile_pool(name="ph2", bufs=1) as ph2,
                tc.tile_pool(name="att1", bufs=1) as att1,
                tc.tile_pool(name="scr2", bufs=2) as scr2,
                tc.tile_pool(name="scr2a", bufs=1) as scr2a,
            ):
                wq_s, wk_s, wv_s, wo_s = wqkvo
                bf = mybir.dt.bfloat16
                qT = ph2.tile([128, ND * S], bf)
                kT = ph2.tile([128, ND * S], bf)
                attnT = mid.tile([128, ND * S], f32, tag="attn")
                vb2 = att1.tile([128, H * DP], bf)

                with tc.tile_pool(name="psqkv", bufs=2, space="PSUM") as psq:
                    for w_s, bcol, dst in ((wq_s, bq_c, qT), (wk_s, bk_c, kT)):
                        for mt in range(ND):
                            ps = psq.tile([128, S], f32, tag="big")
                            for nn in range(2):
                                for k in range(ND):
                                    mm(ps[:, nn * 512:(nn + 1) * 512],
                                       w_s[:, k * D + mt * 128:k * D + (mt + 1) * 128],
                                       seasT[:, k * S + nn * 512:k * S + (nn + 1) * 512],
                                       start=(k == 0), stop=(k == ND - 1))
                            nc.scalar.activation(
                                dst[:, mt * S:(mt + 1) * S], ps[:],
                                Act.Identity, bias=bcol[:, mt:mt + 1], scale=1.0)
                    # v at lags 0..63 only, stacked twice over partitions so
                    # both halves of a head-pair block-diagonal lhsT exist
                    vsrc = att1.tile([128, ND * 128], f32)
                    for k in range(ND):
                        for half in range(2):
                            nc.vector.tensor_copy(
                                r(vsrc[:, k * 128 + 64 * half:k * 128 + 64 * half + 64]),
                                seasT[:, k * S:k * S + DP])
                    ps = psq.tile([128, D], f32, tag="vtok")
                    for k in range(ND):
                        mm(ps[:], vsrc[:, k * 128:(k + 1) * 128],
                           wv_s[:, k * D:(k + 1) * D],
                           start=(k == 0), stop=(k == ND - 1))
                    nc.scalar.activation(vb2[:], ps[:], Act.Copy)
                    # gate the big FFN weight loads on phase-2 progress: the
                    # sync queue stalls on this tiny vb2-dependent copy, so
                    # w1/w2 don't contend with x/wq/wk DMA up front
                    gate = att1.tile([1, 1], bf)
                    nc.sync.dma_start(gate[:], vb2[0:1, 0:1])
                    w1_s = wload(w1p, w1_d, D, DH)
                    w2_s = wload(w2p, w2_d, DH, D)

                # W = Vsum - Vhead = (sum_{s>=64} seas.T[:,s]) @ wv
                W_sb = att1.tile([128, ND], f32)
                sdif = att1.tile([128, 2 * ND], f32)
                for k in range(ND):
                    with nc.allow_low_precision(reason="f32r-typed sum"):
                        nc.vector.tensor_reduce(
                            r(sdif[:, 2 * k:2 * k + 1]),
                            seasT[:, k * S + DP:(k + 1) * S],
                            mybir.AxisListType.X, Alu.add)
                    nc.vector.tensor_copy(r(sdif[:, 2 * k + 1:2 * k + 2]),
                                          sdif[:, 2 * k:2 * k + 1])
                with tc.tile_pool(name="psw", bufs=1, space="PSUM") as psw:
                    ps_w2 = psw.tile([128, 2 * ND], f32, tag="w2")
                    for mt in range(ND):
                        for k in range(ND):
                            mm(ps_w2[:, 2 * mt:2 * mt + 2],
                               wv_s[:, k * D + mt * 128:k * D + (mt + 1) * 128],
                               sdif[:, 2 * k:2 * k + 2],
                               start=(k == 0), stop=(k == ND - 1))
                    nc.vector.tensor_copy(W_sb[:],
                                          ps_w2[:].rearrange("p (a b) -> p a b", b=2)[:, :, 0])

                # E_all pair-major: block p = heads (2p, 2p+1) at rows (0:64,
                # 64:128), cols = tokens
                E_all = att1.tile([128, 4 * S], bf)

                # phase A: packed DFT -> 2 products -> packed inverse -> E
                with (
                    tc.tile_pool(name="psF", bufs=2, space="PSUM") as psF,
                    tc.tile_pool(name="psC", bufs=2, space="PSUM") as psC,
                ):
                    for p in range(4):
                        for nn in range(2):
                            c0 = p * S + nn * 512
                            cr = psC.tile([128, 512], f32, tag="corr")
                            for e in range(2):
                                ro = 64 * e
                                fq_l = fqd[ro:ro + 64, :]
                                fk2_l = fk2d[ro:ro + 64, :]
                                qh = qT[ro:ro + 64, c0:c0 + 512]
                                kh = kT[ro:ro + 64, c0:c0 + 512]
                                qf = psF.tile([128, 512], f32, tag="qf")
                                k1 = psF.tile([128, 512], f32, tag="k1")
                                k2 = psF.tile([128, 512], f32, tag="k2")
                                nc.tensor.matmul(qf[:], fq_l, qh,
                                                 start=True, stop=True)
                                nc.tensor.matmul(k1[:], fq_l, kh,
                                                 start=True, stop=True)
                                nc.tensor.matmul(k2[:], fk2_l, kh,
                                                 start=True, stop=True)
                                qf_sb = scr2a.tile([128, 512], bf, tag="qfsb")
                                nc.scalar.copy(qf_sb[:], qf[:])
                                ab = scr2.tile([128, S], bf, tag="ab")
                                nc.vector.tensor_tensor(ab[:, 0:512], qf_sb[:],
                                                        k1[:], Alu.mult)
                                nc.vector.tensor_tensor(ab[:, 512:1024],
                                                        qf_sb[:], k2[:],
                                                        Alu.mult)
                                nc.tensor.matmul(cr[ro:ro + 64, :], ia_b[:],
                                                 ab[:, 0:512],
                                                 start=True, stop=False)
                                nc.tensor.matmul(cr[ro:ro + 64, :], ib_b[:],
                                                 ab[:, 512:1024],
                                                 start=False, stop=True)
                            nc.scalar.activation(E_all[:, c0:c0 + 512],
                                                 cr[:], Act.Exp, bias=0.0,
                                                 scale=1.0)

                # phase B: Z rows -> 1/Z, then per pair AV + broadcast + fuse
                zsum = att1.tile([8, S], f32)
                zinv_b = att1.tile([8, S], bf)
                with tc.tile_pool(name="psZ", bufs=1, space="PSUM") as psZ:
                    Zall = psZ.tile([8, S], f32, tag="zall")
                    for nn in range(2):
                        sl = slice(nn * 512, (nn + 1) * 512)
                        for p in range(4):
                            nc.tensor.matmul(
                                Zall[:, sl], zblk[:, 8 * p:8 * p + 8],
                                E_all[:, p * S + nn * 512:p * S + (nn + 1) * 512],
                                start=(p == 0), stop=(p == 3))
                    nc.vector.tensor_scalar(zsum[:], Zall[:], float(S - DP),
                                            None, Alu.add)
                with nc.allow_low_precision(reason="1/Z in bf16 is ample"):
                    nc.vector.reciprocal(zinv_b[:], zsum[:])
                # block-diagonal [v_even | v_odd] lhsT per pair
                vblk = att1.tile([128, 4 * 128], bf)
                nc.gpsimd.memset(vblk[:], 0.0)
                for p in range(4):
                    nc.vector.tensor_copy(
                        vblk[0:64, p * 128:p * 128 + 64],
                        vb2[0:64, 128 * p:128 * p + 64])
                    nc.vector.tensor_copy(
                        vblk[64:128, p * 128 + 64:(p + 1) * 128],
                        vb2[64:128, 128 * p + 64:128 * p + 128])
                with (
                    tc.tile_pool(name="psB2", bufs=2, space="PSUM") as psB2,
                    tc.tile_pool(name="psZb", bufs=2, space="PSUM") as psZb,
                ):
                    for nn in range(2):
                        for p in range(4):
                            sl = slice(nn * 512, (nn + 1) * 512)
                            nv = psB2.tile([128, 512], f32, tag="nv")
                            nc.tensor.matmul(
                                nv[:], vblk[:, p * 128:(p + 1) * 128],
                                E_all[:, p * S + nn * 512:p * S + (nn + 1) * 512],
                                start=True, stop=True)
                            zbc = psZb.tile([128, 512], f32, tag="zbc")
                            nc.tensor.matmul(zbc[:],
                                             hsel_b[:, p * 128:(p + 1) * 128],
                                             zinv_b[:, sl],
                                             start=True, stop=True)
                            zbc_sb = scr2a.tile([128, 512], bf, tag="zbcsb")
                            nc.scalar.copy(zbc_sb[:], zbc[:])
                            nc.vector.scalar_tensor_tensor(
                                r(attnT[:, p * S + nn * 512:p * S + (nn + 1) * 512]),
                                nv[:], W_sb[:, p:p + 1], zbc_sb[:],
                                Alu.add, Alu.mult)

            wat_ctx.__exit__(None, None, None)
            out1T_ = out1T

            # ============ layernorm helper (feature-major, per token-half) ==
            def layernorm_T(psln, scr, scr1, src, dst, g_c, be_c, h0):
                if True:
                    s1 = psln.tile([1, 512], f32, tag="stat1")
                    s2 = psln.tile([1, 512], f32, tag="stat2")
                    s1, s2 = s1[:], s2[:]
                    for k in range(ND):
                        c0 = k * S + h0
                        sq = scr.tile([128, 512], f32, tag="lnsq")
                        sq_eng = nc.gpsimd if k % 2 == 0 else nc.vector
                        sq_eng.tensor_tensor(r(sq[:]), src[:, c0:c0 + 512],
                                             src[:, c0:c0 + 512], Alu.mult)
                        mm(s1, ones[:, 0:1], src[:, c0:c0 + 512],
                           start=(k == 0), stop=(k == ND - 1))
                        mm(s2, ones[:, 0:1], sq[:],
                           start=(k == 0), stop=(k == ND - 1))
                    mean = scr1.tile([1, 512], f32, tag="lnm")
                    msq = scr1.tile([1, 512], f32, tag="lnq")
                    var = scr1.tile([1, 512], f32, tag="lnv")
                    sd = scr1.tile([1, 512], f32, tag="lnq")
                    rstd = scr1.tile([1, 512], f32, tag="lnr")
                    bbn = scr1.tile([1, 512], f32, tag="lnv")
                    bb = scr1.tile([1, 512], f32, tag="lnm")
                    nc.vector.tensor_scalar(mean[:], s1, 1.0 / D, None,
                                            Alu.mult)
                    nc.vector.tensor_scalar(var[:], s2, 1.0 / D, EPS,
                                            Alu.mult, Alu.add)
                    nc.vector.tensor_tensor(msq[:], mean[:], mean[:], Alu.mult)
                    nc.vector.tensor_tensor(var[:], var[:], msq[:],
                                            Alu.subtract)
                    nc.scalar.activation(sd[:], var[:], Act.Ln, bias=0.0,
                                         scale=1.0)
                    nc.scalar.activation(r(rstd[:]), sd[:], Act.Exp, bias=0.0,
                                         scale=-0.5)
                    nc.vector.tensor_scalar(bbn[:], mean[:], -1.0, None,
                                            Alu.mult)
                    nc.vector.tensor_tensor(r(bb[:]), bbn[:], rstd[:], Alu.mult)
                    ab2 = psln.tile([128, 1024], f32, tag="lnAB")
                    mm(ab2[:, 0:512], ones[0:1, :], rstd[:])
                    mm(ab2[:, 512:1024], ones[0:1, :], bb[:])
                    for k in range(ND):
                        c0 = k * S + h0
                        t = scr.tile([128, 512], f32, tag="lnt")
                        nc.vector.tensor_tensor(t[:], src[:, c0:c0 + 512],
                                                ab2[:, 0:512], Alu.mult)
                        nc.vector.tensor_tensor(t[:], t[:], ab2[:, 512:1024],
                                                Alu.add)
                        nc.scalar.activation(r(dst[:, c0:c0 + 512]), t[:],
                                             Act.Identity,
                                             bias=be_c[:, k:k + 1],
                                             scale=g_c[:, k:k + 1])

            # ============ back end: wo -> LN1 -> FFN -> LN2 -> LN3; the two
            # token-halves are emitted stage-interleaved so the PE can run
            # half B's matmuls while DVE/Act finish half A's layernorm.
            with (
                tc.tile_pool(name="ph3", bufs=1) as ph3,
                tc.tile_pool(name="scr3", bufs=2) as scr3,
                tc.tile_pool(name="scr3s", bufs=1) as scr3s,
                tc.tile_pool(name="hTp", bufs=1) as hTp,
                tc.tile_pool(name="psbig", bufs=4, space="PSUM") as psbig,
                tc.tile_pool(name="psst", bufs=1, space="PSUM") as psst,
            ):
                sum2T = attnT
                g3p = psbig.tile([128, D], f32, tag="big")
                be3p = psbig.tile([128, D], f32, tag="big")
                mm(g3p[:], ones[0:1, :], g3_r[:])
                mm(be3p[:], ones[0:1, :], be3_r[:])
                g3bc = scr3s.tile([128, D], f32, tag="g3bc")
                be3bc = scr3s.tile([128, D], f32, tag="be3bc")
                nc.vector.tensor_copy(g3bc[:], g3p[:])
                nc.vector.tensor_copy(be3bc[:], be3p[:])
                mv_all = scr3s.tile([128, 2 * NT], f32, tag="st3mv")
                rstd3 = scr3s.tile([128, NT], f32, tag="st3r")
                nb3 = scr3s.tile([128, NT], f32, tag="st3nb")

                def stage_wo(hf):
                    h0 = hf * 512
                    for mt in range(ND):
                        ps = psbig.tile([128, 512], f32, tag="big")
                        for k in range(ND):
                            mm(ps[:],
                               wo_s[:, k * D + mt * 128:k * D + (mt + 1) * 128],
                               attnT[:, k * S + h0:k * S + h0 + 512],
                               start=(k == 0), stop=(k == ND - 1))
                        nc.vector.scalar_tensor_tensor(
                            r(out1T_[:, mt * S + h0:mt * S + h0 + 512]),
                            ps[:], bo_c[:, mt:mt + 1],
                            seasT[:, mt * S + h0:mt * S + h0 + 512],
                            Alu.add, Alu.add)

                def stage_ln1(hf):
                    layernorm_T(psst, scr3, scr3s, out1T_, out1T_,
                                g1_c, be1_c, hf * 512)

                def stage_ffn1(hf):
                    h0 = hf * 512
                    hTa = hTp.tile([128, 8 * 512], f32, tag="hTa")
                    hTb = hTp.tile([128, 8 * 512], f32, tag="hTb")
                    hTs[hf] = [hTa, hTb]
                    for mt in range(NH):
                        ps = psbig.tile([128, 512], f32, tag="big")
                        for k in range(ND):
                            mm(ps[:],
                               w1_s[:, k * DH + mt * 128:k * DH + (mt + 1) * 128],
                               out1T_[:, k * S + h0:k * S + h0 + 512],
                               start=(k == 0), stop=(k == ND - 1))
                        ht_dst = hTs[hf][mt // 8][:, (mt % 8) * 512:(mt % 8 + 1) * 512]
                        if mt % 2 == 0:
                            nc.scalar.activation(
                                r(ht_dst), ps[:],
                                Act.Relu, bias=b1_c[:, mt:mt + 1], scale=1.0)
                        else:
                            nc.vector.tensor_scalar(
                                r(ht_dst), ps[:],
                                b1_c[:, mt:mt + 1], 0.0, Alu.add, Alu.max)

                def stage_ffn2(hf):
                    h0 = hf * 512
                    for mt in range(ND):
                        ps = psbig.tile([128, 512], f32, tag="big")
                        for k in range(NH):
                            mm(ps[:],
                               w2_s[:, k * D + mt * 128:k * D + (mt + 1) * 128],
                               hTs[hf][k // 8][:, (k % 8) * 512:(k % 8 + 1) * 512],
                               start=(k == 0), stop=(k == NH - 1))
                        nc.vector.scalar_tensor_tensor(
                            r(sum2T[:, mt * S + h0:mt * S + h0 + 512]),
                            ps[:], b2_c[:, mt:mt + 1],
                            out1T_[:, mt * S + h0:mt * S + h0 + 512],
                            Alu.add, Alu.add)

                def stage_ln2(hf):
                    # x_out = LN2_out + seas + trend = LN2_out + x, so the
                    # seasonal residual folds into the final x_tok add
                    layernorm_T(psst, scr3, scr3s, sum2T, sum2T,
                                g2_c, be2_c, hf * 512)

                def stage_final(hf):
                    # transpose back + x add; LN3 stats via bn_stats; the 4
                    # token tiles share one contiguous buffer so the final
                    # store is a single DMA trigger
                    xos = []
                    for i, st in enumerate(range(hf * 4, hf * 4 + 4)):
                        ps = psbig.tile([128, 512], f32, tag="big")
                        for kt in range(ND):
                            nc.tensor.transpose(
                                ps[:, kt * 128:(kt + 1) * 128],
                                sum2T[:, kt * S + st * 128:kt * S + (st + 1) * 128],
                                ident[:])
                        xo_st = ph3.tile([128, D], f32, tag=f"xot{st % 4}")
                        xos.append(xo_st)
                        nc.vector.tensor_tensor(
                            xo_st[:], ps[:], x_tok[:, st * D:(st + 1) * D],
                            Alu.add)
                        bns = scr3.tile([128, 6], f32, tag="bns")
                        nc.vector.bn_stats(bns[:], xo_st[:])
                        nc.vector.bn_aggr(mv_all[:, 2 * st:2 * st + 2], bns[:])
                    hs = slice(hf * 4, hf * 4 + 4)
                    mv = mv_all[:].rearrange("p (s two) -> p s two", two=2)
                    means = mv[:, hs, 0]
                    varis = mv[:, hs, 1]
                    nc.vector.tensor_scalar(rstd3[:, hs], varis, EPS, None,
                                            Alu.add)
                    nc.scalar.activation(rstd3[:, hs], rstd3[:, hs], Act.Ln,
                                         bias=0.0, scale=1.0)
                    nc.scalar.activation(rstd3[:, hs], rstd3[:, hs], Act.Exp,
                                         bias=0.0, scale=-0.5)
                    nc.vector.scalar_tensor_tensor(
                        nb3[:, hs], means, -1.0, rstd3[:, hs],
                        Alu.mult, Alu.mult)
                    for i, st in enumerate(range(hf * 4, hf * 4 + 4)):
                        xn = scr3.tile([128, D], f32, tag="xn")
                        nc.scalar.activation(
                            xn[:], xos[i][:],
                            Act.Identity, bias=nb3[:, st:st + 1],
                            scale=rstd3[:, st:st + 1])
                        nc.vector.tensor_tensor(xn[:], xn[:], g3bc[:],
                                                Alu.mult)
                        nc.gpsimd.tensor_tensor(xn[:], xn[:], be3bc[:],
                                                Alu.add)
                        nc.sync.dma_start(out_d[st * 128:(st + 1) * 128, :],
                                          xn[:])

                hTs = {}
                stage_wo(0)
                stage_ln1(0)
                stage_wo(1)
                stage_ffn1(0)
                stage_ln1(1)
                stage_ffn2(0)
                stage_ffn1(1)
                stage_ln2(0)
                stage_ffn2(1)
                stage_final(0)
                stage_ln2(1)
                stage_final(1)

            w2_ctx.__exit__(None, None, None)
            w1_ctx.__exit__(None, None, None)
            wop_p.__exit__(None, None, None)

    nc.compile()
    return nc


def _get_nc():
    if "nc" not in _CACHE:
        _CACHE["nc"] = _build()
    return _CACHE["nc"]


def kernel(**inputs):
    from concourse.bass_utils import run_bass_kernel_spmd

    nc = _get_nc()
    names = ["wq", "bq", "wk", "bk", "wv", "wo", "bo", "w1", "b1",
             "w2", "b2", "g1", "be1", "g2", "be2", "g3", "be3"]
    shared = {k: np.ascontiguousarray(np.asarray(inputs[k], np.float32))
              for k in names}
    # attention weights sum to 1 per row, so the v-bias passes through the
    # weighted average exactly: fold bv@wo into bo.
    bv = np.asarray(inputs["bv"], np.float64)
    wo = np.asarray(inputs["wo"], np.float64)
    shared["bo"] = np.ascontiguousarray(
        (np.asarray(inputs["bo"], np.float64) + bv @ wo).astype(np.float32))
    x = np.ascontiguousarray(np.asarray(inputs["x"], np.float32))
    in_maps = [dict(shared, x=x[b]) for b in range(NCORES)]
    res = run_bass_kernel_spmd(nc, in_maps, list(range(NCORES)))
    out = np.stack([res.results[b]["out"] for b in range(NCORES)], axis=0)
    return out.astype(np.float32)



# revision 17
# speedup vs baseline: 1.0570x; 1.0570x over previous
"""Autoformer encoder block on 8 TRN2 NeuronCores.

Sharding: data-parallel over batch (B=8 -> 1 batch per core), weights
replicated. No collectives.

Per-core math (S=1024, D=512, H=8, dp=64, K=25):
  trend = movavg(x)               # banded matmul, token-major
  seas  = x - trend               # token-major, then PE-transpose -> seas.T
  q.T/k.T = wq/wk.T @ seas.T      # feature-major
  v     = seas @ wv               # token-major (for AV lhsT + V-sums)
  The reference's rfft/irfft over the depth axis (n=2S) makes
  corr[b,h,s,t] == 0 for t >= dp, so attention reduces to 64 depth-lags:
    corr.T = IDFT @ (QF (*) conj(KF)), QF = FWD.T @ q.T   (n=128 DFT)
    E = exp(corr/8); out = (E @ v[:64] + (Vsum - Vhead)) / (rowsum(E)+S-dp)
  wo, LN1, FFN(4x, relu), LN2 feature-major (stats via ones-matmul).
  seasonal_out + trend == x_out exactly (trend2 cancels), so movavg2 is
  skipped; final LN3 runs token-major after a PE-transpose, then DMA out.

All heavy matmuls run in bf16 (weights cast+packed host-side into a few
contiguous [128, N] DRAM blobs so each loads with ONE DMA trigger);
activations between stages are bf16 as well.  Residuals/statistics stay
f32.  DMA triggers are spread across the three DMA-capable queues
(sync/scalar/gpsimd) with the critical x+band loads first.
"""

import numpy as np

B, S, D, H = 8, 1024, 512, 8
DP = D // H
DH = 4 * D
KWIN, PAD = 25, 12
EPS = 1e-6
NCORES = 8
NT = S // 128   # 8 token tiles
ND = D // 128   # 4 feature tiles
NH = DH // 128  # 16 hidden tiles

_CACHE = {}

# CF32 pack layout (f32): BM | BPN | ones | hsel
CF_BM, CF_BPN, CF_ONES, CF_HSEL = 0, 1024, 2048, 2176
CF_W = 2176 + 512
# CBF pack layout (bf16): fqd | fk2d | ia | ib | zblk | identb | onesb | hsel
CB_FQD, CB_FK2, CB_IA, CB_IB, CB_ZBLK, CB_ID, CB_ONE, CB_HSEL = (
    0, 128, 256, 320, 384, 416, 544, 672)
CB_W = 672 + 512
# colpack layout: bq | bk | bo | b2 | b1 | g1 | be1 | g2 | be2
CO_BQ, CO_BK, CO_BO, CO_B2, CO_B1, CO_G1, CO_BE1, CO_G2, CO_BE2 = (
    0, 4, 8, 12, 16, 32, 36, 40, 44)
CO_W = 48


def _consts():
    c = {}
    # moving-average 3-piece band blocks over token-major x tiles:
    # trend tile j = BM[:,j]^T x[j] + BP[:,j]^T x[j-1][116:128] + BN[:,j]^T x[j+1][0:12]
    cnt = np.minimum(S, np.arange(S) + PAD + 1) - np.maximum(0, np.arange(S) - PAD)
    BM = np.zeros((128, NT * 128), np.float32)
    # BPN rows 0:12 = next-tile piece, rows 64:128 = prev-tile piece
    BPN = np.zeros((128, NT * 128), np.float32)
    for j in range(NT):
        for cc in range(128):
            s = 128 * j + cc
            for i in range(128):
                if abs(i - cc) <= PAD:
                    BM[i, 128 * j + cc] = 1.0 / cnt[s]
            for i in range(64, 128):
                if j >= 1 and abs(128 * (j - 1) + i - s) <= PAD:
                    BPN[i, 128 * j + cc] = 1.0 / cnt[s]
            for i in range(PAD):
                if j < NT - 1 and abs(128 * (j + 1) + i - s) <= PAD:
                    BPN[i, 128 * j + cc] = 1.0 / cnt[s]

    import ml_dtypes
    bf = ml_dtypes.bfloat16

    # head-pair selector for 1/Z broadcast: pass p covers heads 2p, 2p+1
    HSEL = np.zeros((128, 4 * 128), np.float32)
    for p in range(4):
        for mm_ in range(128):
            HSEL[2 * p + mm_ // 64, p * 128 + mm_] = 1.0

    cf32 = np.zeros((128, CF_W), np.float32)
    cf32[:, CF_BM:CF_BM + 1024] = BM
    cf32[:, CF_BPN:CF_BPN + 1024] = BPN
    cf32[:, CF_ONES:CF_ONES + 128] = 1.0
    cf32[:, CF_HSEL:CF_HSEL + 512] = HSEL
    c["CF32"] = cf32

    # packed forward DFT (n=128): FQ [64,128] = [cos f=0..64 | sin f=1..63],
    # FK2 [64,128] = [sin f=0..64 | cos f=1..63]; doubled over partitions so
    # heads at base 0 and base 64 can slice the same constant.
    n = 2 * DP
    d = np.arange(DP)[:, None]
    f65 = np.arange(65)[None, :]
    f63 = np.arange(1, 64)[None, :]
    FQ = np.concatenate([np.cos(2 * np.pi * f65 * d / n),
                         np.sin(2 * np.pi * f63 * d / n)], axis=1)
    FK2 = np.concatenate([np.sin(2 * np.pi * f65 * d / n),
                          np.cos(2 * np.pi * f63 * d / n)], axis=1)

    # packed inverse (softmax scale 1/sqrt(dp) folded in):
    # corr'[t] = IA^T @ (qf*k1f) + IB^T @ (qf*k2f)
    t = np.arange(DP)[None, :]
    w = np.full(65, 2.0); w[0] = 1.0; w[64] = 1.0
    fc = np.arange(65)[:, None]
    fs = np.arange(1, 64)[:, None]
    scale = 1.0 / np.sqrt(DP)
    IA = np.concatenate([(w[:, None] / n) * np.cos(2 * np.pi * fc * t / n),
                         (2.0 / n) * np.cos(2 * np.pi * fs * t / n)], axis=0)
    IB = np.concatenate([-(w[:, None] / n) * np.sin(2 * np.pi * fc * t / n),
                         (2.0 / n) * np.sin(2 * np.pi * fs * t / n)], axis=0)

    # Z rows: ZBLK[:, 8p+j] sums E-pair rows 0:64 into row 2p, 64:128 into 2p+1
    ZBLK = np.zeros((128, 32), np.float32)
    for p in range(4):
        ZBLK[0:64, 8 * p + 2 * p] = 1.0
        ZBLK[64:128, 8 * p + 2 * p + 1] = 1.0

    cbf = np.zeros((128, CB_W), np.float32)
    cbf[:, CB_FQD:CB_FQD + 128] = np.concatenate([FQ, FQ], axis=0)
    cbf[:, CB_FK2:CB_FK2 + 128] = np.concatenate([FK2, FK2], axis=0)
    cbf[:, CB_IA:CB_IA + DP] = scale * IA
    cbf[:, CB_IB:CB_IB + DP] = scale * IB
    cbf[:, CB_ZBLK:CB_ZBLK + 32] = ZBLK
    cbf[:, CB_ID:CB_ID + 128] = np.eye(128)
    cbf[:, CB_ONE:CB_ONE + 128] = 1.0
    cbf[:, CB_HSEL:CB_HSEL + 512] = HSEL
    c["CBF"] = cbf.astype(bf)
    return c


def _build():
    import concourse.bacc as bacc
    import concourse.mybir as mybir
    import concourse.tile as tile

    f32 = mybir.dt.float32
    f32r = mybir.dt.float32r
    bfdt = mybir.dt.bfloat16
    Alu = mybir.AluOpType
    Act = mybir.ActivationFunctionType

    nc = bacc.Bacc()

    # ---- DRAM parameters (host-packed) ----
    x_d = nc.declare_dram_parameter("x", [S, D], f32, isOutput=False)
    wqkvo_d = nc.declare_dram_parameter("wqkvo", [128, 4 * ND * D], bfdt,
                                        isOutput=False)
    w12_d = nc.declare_dram_parameter("w12", [128, 2 * ND * DH], bfdt,
                                      isOutput=False)
    colp_d = nc.declare_dram_parameter("colpack", [128, CO_W], f32,
                                       isOutput=False)
    g3be3_d = nc.declare_dram_parameter("g3be3", [1, 2 * D], f32,
                                        isOutput=False)
    out_d = nc.declare_dram_parameter("out", [S, D], f32, isOutput=True)

    C = _consts()
    CF32_d = nc.inline_tensor(C["CF32"], name="c_f32")
    CBF_d = nc.inline_tensor(C["CBF"], name="c_bf")

    def r(ap):
        return ap.bitcast(f32r)

    def mm(out, lhsT, rhs, start=True, stop=True):
        nc.tensor.matmul(out, r(lhsT), r(rhs), start=start, stop=stop)

    def mmb(out, lhsT, rhs, start=True, stop=True):
        nc.tensor.matmul(out, lhsT, rhs, start=start, stop=stop)

    with tile.TileContext(nc) as tc:
        with (
            tc.tile_pool(name="konst", bufs=1) as konst,
            tc.tile_pool(name="persist", bufs=1) as persist,
            tc.tile_pool(name="mid", bufs=1) as mid,
        ):
            # ---- packed consts to SBUF (few triggers, critical first) ----
            cf32 = konst.tile([128, CF_W], f32)
            cbf = konst.tile([128, CB_W], bfdt)
            colp = konst.tile([128, CO_W], f32)
            g3be3_r = konst.tile([1, 2 * D], f32)
            x_tok = persist.tile([128, NT * D], f32)
            seasT = persist.tile([128, ND * S], bfdt)

            nc.sync.dma_start(r(cf32[:]), r(CF32_d[:]))
            for g_ in range(2):
                nc.sync.dma_start(
                    r(x_tok[:, g_ * 4 * D:(g_ + 1) * 4 * D].rearrange(
                        "p (st d) -> p st d", d=D)),
                    r(x_d[g_ * 512:(g_ + 1) * 512, :].rearrange(
                        "(st p) d -> p st d", p=128)))
            nc.scalar.dma_start(cbf[:], CBF_d[:])
            nc.scalar.dma_start(colp[:], colp_d[:])
            nc.scalar.dma_start(r(g3be3_r[:]), r(g3be3_d[:]))

            bandm = cf32[:, CF_BM:CF_BM + 1024]
            bandpn = cf32[:, CF_BPN:CF_BPN + 1024]
            ones = cf32[:, CF_ONES:CF_ONES + 128]
            hsel = cbf[0:8, CB_HSEL:CB_HSEL + 512]
            fqd = cbf[:, CB_FQD:CB_FQD + 128]
            fk2d = cbf[:, CB_FK2:CB_FK2 + 128]
            ia_b = cbf[:, CB_IA:CB_IA + DP]
            ib_b = cbf[:, CB_IB:CB_IB + DP]
            zblk = cbf[:, CB_ZBLK:CB_ZBLK + 32]
            identb = cbf[:, CB_ID:CB_ID + 128]
            onesb = cbf[:, CB_ONE:CB_ONE + 128]
            bq_c = colp[:, CO_BQ:CO_BQ + ND]
            bk_c = colp[:, CO_BK:CO_BK + ND]
            bo_c = colp[:, CO_BO:CO_BO + ND]
            b2_c = colp[:, CO_B2:CO_B2 + ND]
            b1_c = colp[:, CO_B1:CO_B1 + NH]
            g1_c = colp[:, CO_G1:CO_G1 + ND]
            be1_c = colp[:, CO_BE1:CO_BE1 + ND]
            g2_c = colp[:, CO_G2:CO_G2 + ND]
            be2_c = colp[:, CO_BE2:CO_BE2 + ND]
            g3_r = g3be3_r[0:1, 0:D]
            be3_r = g3be3_r[0:1, D:2 * D]

            # weight blobs: wqkvo early on scalar queue; w12 gated on
            # movavg completion (gpsimd queue) so x keeps HBM priority
            w12_ctx = tc.tile_pool(name="w12p", bufs=1)
            w12p = w12_ctx.__enter__()
            wat_ctx = tc.tile_pool(name="wat", bufs=1)
            wat = wat_ctx.__enter__()
            wqkvo_s = wat.tile([128, 4 * ND * D], bfdt)
            nc.scalar.dma_start(wqkvo_s[:], wqkvo_d[:])
            wq_s = wqkvo_s[:, 0 * ND * D:1 * ND * D]
            wk_s = wqkvo_s[:, 1 * ND * D:2 * ND * D]
            wv_s = wqkvo_s[:, 2 * ND * D:3 * ND * D]
            wo_s = wqkvo_s[:, 3 * ND * D:4 * ND * D]
            w12_s = w12p.tile([128, 2 * ND * DH], bfdt)
            w1_s = w12_s[:, 0:ND * DH]
            w2_s = w12_s[:, ND * DH:2 * ND * DH]

            # ============ phase 1: load x, moving average, transpose ========
            with (
                tc.tile_pool(name="ph1", bufs=1) as ph1,
                tc.tile_pool(name="ps1", bufs=2, space="PSUM") as ps1,
            ):
                seas_tok = ph1.tile([128, NT * D], bfdt)
                for j in range(NT):
                    ps = ps1.tile([128, D], f32, tag="mavg")
                    mm(ps[:], bandm[:, 128 * j:128 * (j + 1)],
                       x_tok[:, j * D:(j + 1) * D], start=True, stop=False)
                    if j >= 1:
                        mm(ps[:], bandpn[64:128, 128 * j:128 * (j + 1)],
                           x_tok[64:128, (j - 1) * D:j * D],
                           start=False, stop=(j == NT - 1))
                    if j < NT - 1:
                        mm(ps[:], bandpn[0:PAD, 128 * j:128 * (j + 1)],
                           x_tok[0:PAD, (j + 1) * D:(j + 2) * D],
                           start=False, stop=True)
                    nc.vector.tensor_tensor(
                        seas_tok[:, j * D:(j + 1) * D],
                        x_tok[:, j * D:(j + 1) * D], ps[:], Alu.subtract)

                # w12 load gated on movavg progress (gpsimd DMA queue)
                gate = ph1.tile([1, 1], bfdt)
                nc.gpsimd.dma_start(gate[:], seas_tok[0:1, 3 * D:3 * D + 1])
                nc.gpsimd.dma_start(w12_s[:], w12_d[:])

                for st in range(NT):
                    for kt in range(ND):
                        ps = ps1.tile([128, 128], bfdt, tag="tr")
                        nc.tensor.transpose(
                            ps[:],
                            seas_tok[:, st * D + kt * 128:st * D + (kt + 1) * 128],
                            identb)
                        dst = seasT[:, kt * S + st * 128:kt * S + (st + 1) * 128]
                        if (st + kt) % 2 == 0:
                            nc.vector.tensor_copy(dst, ps[:])
                        else:
                            nc.scalar.activation(dst, ps[:], Act.Identity,
                                                 bias=0.0, scale=1.0)

            # ============ phase 2: attention ============
            out1T = mid.tile([128, ND * S], bfdt, tag="m16")
            with (
                tc.tile_pool(name="ph2", bufs=1) as ph2,
                tc.tile_pool(name="att1", bufs=1) as att1,
                tc.tile_pool(name="scr2", bufs=2) as scr2,
                tc.tile_pool(name="scr2a", bufs=1) as scr2a,
            ):
                qT = ph2.tile([128, ND * S], bfdt)
                kT = ph2.tile([128, ND * S], bfdt)
                attnT = mid.tile([128, ND * S], bfdt, tag="attn")
                vb2 = att1.tile([128, H * DP], bfdt)

                with tc.tile_pool(name="psqkv", bufs=2, space="PSUM") as psq:
                    for w_s, bcol, dst in ((wq_s, bq_c, qT), (wk_s, bk_c, kT)):
                        for mt in range(ND):
                            ps = psq.tile([128, S], f32, tag="big")
                            for nn in range(2):
                                for k in range(ND):
                                    mmb(ps[:, nn * 512:(nn + 1) * 512],
                                        w_s[:, k * D + mt * 128:k * D + (mt + 1) * 128],
                                        seasT[:, k * S + nn * 512:k * S + (nn + 1) * 512],
                                        start=(k == 0), stop=(k == ND - 1))
                            nc.scalar.activation(
                                dst[:, mt * S:(mt + 1) * S], ps[:],
                                Act.Identity, bias=bcol[:, mt:mt + 1], scale=1.0)
                    # v at lags 0..63 only, stacked twice over partitions so
                    # both halves of a head-pair block-diagonal lhsT exist
                    vsrc = att1.tile([128, ND * 128], bfdt)
                    for k in range(ND):
                        for half in range(2):
                            nc.vector.tensor_copy(
                                vsrc[:, k * 128 + 64 * half:k * 128 + 64 * half + 64],
                                seasT[:, k * S:k * S + DP])
                    ps = psq.tile([128, D], f32, tag="vtok")
                    for k in range(ND):
                        mmb(ps[:], vsrc[:, k * 128:(k + 1) * 128],
                            wv_s[:, k * D:(k + 1) * D],
                            start=(k == 0), stop=(k == ND - 1))
                    nc.scalar.activation(vb2[:], ps[:], Act.Identity,
                                         bias=0.0, scale=1.0)

                # W = Vsum - Vhead = (sum_{s>=64} seas.T[:,s]) @ wv
                W_sb = att1.tile([128, ND], f32)
                sdif = att1.tile([128, 2 * ND], bfdt)
                for k in range(ND):
                    with nc.allow_low_precision(reason="bf16 lag-sum is ample"):
                        nc.vector.tensor_reduce(
                            sdif[:, 2 * k:2 * k + 1],
                            seasT[:, k * S + DP:(k + 1) * S],
                            mybir.AxisListType.X, Alu.add)
                    nc.vector.tensor_copy(sdif[:, 2 * k + 1:2 * k + 2],
                                          sdif[:, 2 * k:2 * k + 1])
                with tc.tile_pool(name="psw", bufs=1, space="PSUM") as psw:
                    ps_w2 = psw.tile([128, 2 * ND], f32, tag="w2")
                    for mt in range(ND):
                        for k in range(ND):
                            mmb(ps_w2[:, 2 * mt:2 * mt + 2],
                                wv_s[:, k * D + mt * 128:k * D + (mt + 1) * 128],
                                sdif[:, 2 * k:2 * k + 2],
                                start=(k == 0), stop=(k == ND - 1))
                    nc.vector.tensor_copy(W_sb[:],
                                          ps_w2[:].rearrange("p (a b) -> p a b", b=2)[:, :, 0])

                # E_all pair-major: block p = heads (2p, 2p+1) at rows (0:64,
                # 64:128), cols = tokens
                E_all = att1.tile([128, 4 * S], bfdt)

                # phase A: packed DFT -> 2 products -> packed inverse -> E
                with (
                    tc.tile_pool(name="psF", bufs=2, space="PSUM") as psF,
                    tc.tile_pool(name="psC", bufs=2, space="PSUM") as psC,
                ):
                    for p in range(4):
                        for nn in range(2):
                            c0 = p * S + nn * 512
                            cr = psC.tile([128, 512], f32, tag="corr")
                            for e in range(2):
                                ro = 64 * e
                                fq_l = fqd[ro:ro + 64, :]
                                fk2_l = fk2d[ro:ro + 64, :]
                                qh = qT[ro:ro + 64, c0:c0 + 512]
                                kh = kT[ro:ro + 64, c0:c0 + 512]
                                qf = psF.tile([128, 512], f32, tag="qf")
                                k1 = psF.tile([128, 512], f32, tag="k1")
                                k2 = psF.tile([128, 512], f32, tag="k2")
                                nc.tensor.matmul(qf[:], fq_l, qh,
                                                 start=True, stop=True)
                                nc.tensor.matmul(k1[:], fq_l, kh,
                                                 start=True, stop=True)
                                nc.tensor.matmul(k2[:], fk2_l, kh,
                                                 start=True, stop=True)
                                qf_sb = scr2a.tile([128, 512], bfdt, tag="qfsb")
                                nc.vector.tensor_copy(qf_sb[:], qf[:])
                                ab = scr2.tile([128, S], bfdt, tag="ab")
                                nc.vector.tensor_tensor(ab[:, 0:512], qf_sb[:],
                                                        k1[:], Alu.mult)
                                nc.vector.tensor_tensor(ab[:, 512:1024],
                                                        qf_sb[:], k2[:],
                                                        Alu.mult)
                                nc.tensor.matmul(cr[ro:ro + 64, :], ia_b[:],
                                                 ab[:, 0:512],
                                                 start=True, stop=False)
                                nc.tensor.matmul(cr[ro:ro + 64, :], ib_b[:],
                                                 ab[:, 512:1024],
                                                 start=False, stop=True)
                            nc.scalar.activation(E_all[:, c0:c0 + 512],
                                                 cr[:], Act.Exp, bias=0.0,
                                                 scale=1.0)

                # phase B: Z rows -> 1/Z, then per pair AV + broadcast + fuse
                zsum = att1.tile([8, S], f32)
                zinv = att1.tile([8, S], f32)
                with tc.tile_pool(name="psZ", bufs=1, space="PSUM") as psZ:
                    Zall = psZ.tile([8, S], f32, tag="zall")
                    for nn in range(2):
                        sl = slice(nn * 512, (nn + 1) * 512)
                        for p in range(4):
                            nc.tensor.matmul(
                                Zall[:, sl], zblk[:, 8 * p:8 * p + 8],
                                E_all[:, p * S + nn * 512:p * S + (nn + 1) * 512],
                                start=(p == 0), stop=(p == 3))
                    nc.vector.tensor_scalar(zsum[:], Zall[:], float(S - DP),
                                            None, Alu.add)
                zinv_b = att1.tile([8, S], bfdt)
                with nc.allow_low_precision(reason="1/Z approx is ample"):
                    nc.vector.reciprocal_approx_fast(out=zinv[:], in_=zsum[:])
                    nc.vector.tensor_copy(zinv_b[:], zinv[:])
                # block-diagonal [v_even | v_odd] lhsT per pair
                vblk = att1.tile([128, 4 * 128], bfdt)
                nc.gpsimd.memset(vblk[:], 0.0)
                for p in range(4):
                    nc.vector.tensor_copy(
                        vblk[0:64, p * 128:p * 128 + 64],
                        vb2[0:64, 128 * p:128 * p + 64])
                    nc.vector.tensor_copy(
                        vblk[64:128, p * 128 + 64:(p + 1) * 128],
                        vb2[64:128, 128 * p + 64:128 * p + 128])
                with (
                    tc.tile_pool(name="psB2", bufs=2, space="PSUM") as psB2,
                    tc.tile_pool(name="psZb", bufs=2, space="PSUM") as psZb,
                ):
                    for nn in range(2):
                        for p in range(4):
                            sl = slice(nn * 512, (nn + 1) * 512)
                            nv = psB2.tile([128, 512], f32, tag="nv")
                            nc.tensor.matmul(
                                nv[:], vblk[:, p * 128:(p + 1) * 128],
                                E_all[:, p * S + nn * 512:p * S + (nn + 1) * 512],
                                start=True, stop=True)
                            zbc = psZb.tile([128, 512], f32, tag="zbc")
                            mmb(zbc[:], hsel[:, p * 128:(p + 1) * 128],
                                zinv_b[:, sl])
                            zbc_sb = scr2a.tile([128, 512], f32, tag="zbcsb")
                            nc.vector.tensor_copy(zbc_sb[:], zbc[:])
                            nc.vector.scalar_tensor_tensor(
                                attnT[:, p * S + nn * 512:p * S + (nn + 1) * 512],
                                nv[:], W_sb[:, p:p + 1], zbc_sb[:],
                                Alu.add, Alu.mult)

            out1T_ = out1T

            # ============ layernorm helper (feature-major, per token-half) ==
            def layernorm_T(psln, scr, scr1, src, dst, g_c, be_c, h0):
                if True:
                    s1 = psln.tile([1, 512], f32, tag="stat1")
                    s2 = psln.tile([1, 512], f32, tag="stat2")
                    s1, s2 = s1[:], s2[:]
                    for k in range(ND):
                        c0 = k * S + h0
                        sq = scr.tile([128, 512], bfdt, tag="lnsq")
                        sq_eng = nc.gpsimd if k % 2 == 0 else nc.vector
                        sq_eng.tensor_tensor(sq[:], src[:, c0:c0 + 512],
                                             src[:, c0:c0 + 512], Alu.mult)
                        mmb(s1, onesb[:, 0:1], src[:, c0:c0 + 512],
                            start=(k == 0), stop=(k == ND - 1))
                        mmb(s2, onesb[:, 0:1], sq[:],
                            start=(k == 0), stop=(k == ND - 1))
                    mean = scr1.tile([1, 512], f32, tag="lnm")
                    msq = scr1.tile([1, 512], f32, tag="lnq")
                    var = scr1.tile([1, 512], f32, tag="lnv")
                    rstd = scr1.tile([1, 512], f32, tag="lnr")
                    bbn = scr1.tile([1, 512], f32, tag="lnv")
                    bb = scr1.tile([1, 512], f32, tag="lnm")
                    nc.vector.tensor_scalar(mean[:], s1, 1.0 / D, None,
                                            Alu.mult)
                    nc.vector.tensor_scalar(var[:], s2, 1.0 / D, EPS,
                                            Alu.mult, Alu.add)
                    nc.vector.tensor_tensor(msq[:], mean[:], mean[:], Alu.mult)
                    nc.vector.tensor_tensor(var[:], var[:], msq[:],
                                            Alu.subtract)
                    nc.scalar.activation(r(rstd[:]), var[:],
                                         Act.Abs_reciprocal_sqrt,
                                         bias=0.0, scale=1.0)
                    nc.vector.tensor_scalar(bbn[:], mean[:], -1.0, None,
                                            Alu.mult)
                    nc.vector.tensor_tensor(r(bb[:]), bbn[:], rstd[:], Alu.mult)
                    ab2 = psln.tile([128, 1024], f32, tag="lnAB")
                    mm(ab2[:, 0:512], ones[0:1, :], rstd[:])
                    mm(ab2[:, 512:1024], ones[0:1, :], bb[:])
                    for k in range(ND):
                        c0 = k * S + h0
                        t = scr.tile([128, 512], f32, tag="lnt")
                        nc.vector.tensor_tensor(t[:], src[:, c0:c0 + 512],
                                                ab2[:, 0:512], Alu.mult)
                        nc.vector.tensor_tensor(t[:], t[:], ab2[:, 512:1024],
                                                Alu.add)
                        nc.scalar.activation(dst[:, c0:c0 + 512], t[:],
                                             Act.Identity,
                                             bias=be_c[:, k:k + 1],
                                             scale=g_c[:, k:k + 1])

            # ============ back end: wo -> LN1 -> FFN -> LN2 -> LN3; the two
            # token-halves are emitted stage-interleaved so the PE can run
            # half B's matmuls while DVE/Act finish half A's layernorm.
            with (
                tc.tile_pool(name="ph3", bufs=1) as ph3,
                tc.tile_pool(name="scr3", bufs=2) as scr3,
                tc.tile_pool(name="scr3s", bufs=1) as scr3s,
                tc.tile_pool(name="hTp", bufs=1) as hTp,
            ):
                psbig_ctx = tc.tile_pool(name="psbig", bufs=3, space="PSUM")
                psbig = psbig_ctx.__enter__()
                psst_ctx = tc.tile_pool(name="psst", bufs=1, space="PSUM")
                psst = psst_ctx.__enter__()
                sum2T = attnT
                g3p = psbig.tile([128, D], f32, tag="big")
                be3p = psbig.tile([128, D], f32, tag="big")
                mm(g3p[:], ones[0:1, :], g3_r)
                mm(be3p[:], ones[0:1, :], be3_r)
                g3bc = scr3s.tile([128, D], f32, tag="g3bc")
                be3bc = scr3s.tile([128, D], f32, tag="be3bc")
                nc.vector.tensor_copy(g3bc[:], g3p[:])
                nc.vector.tensor_copy(be3bc[:], be3p[:])
                mv_all = scr3s.tile([128, 2 * NT], f32, tag="st3mv")
                rstd3 = scr3s.tile([128, NT], f32, tag="st3r")
                nb3 = scr3s.tile([128, NT], f32, tag="st3nb")

                def stage_wo(hf):
                    h0 = hf * 512
                    for mt in range(ND):
                        ps = psbig.tile([128, 512], f32, tag="big")
                        for k in range(ND):
                            mmb(ps[:],
                                wo_s[:, k * D + mt * 128:k * D + (mt + 1) * 128],
                                attnT[:, k * S + h0:k * S + h0 + 512],
                                start=(k == 0), stop=(k == ND - 1))
                        nc.vector.scalar_tensor_tensor(
                            out1T_[:, mt * S + h0:mt * S + h0 + 512],
                            ps[:], bo_c[:, mt:mt + 1],
                            seasT[:, mt * S + h0:mt * S + h0 + 512],
                            Alu.add, Alu.add)

                def stage_ln1(hf):
                    layernorm_T(psst, scr3, scr3s, out1T_, out1T_,
                                g1_c, be1_c, hf * 512)

                def stage_ffn1(hf):
                    h0 = hf * 512
                    hTa = hTp.tile([128, 8 * 512], bfdt, tag="hTa")
                    hTb = hTp.tile([128, 8 * 512], bfdt, tag="hTb")
                    hTs[hf] = [hTa, hTb]
                    for mt in range(NH):
                        ps = psbig.tile([128, 512], f32, tag="big")
                        for k in range(ND):
                            mmb(ps[:],
                                w1_s[:, k * DH + mt * 128:k * DH + (mt + 1) * 128],
                                out1T_[:, k * S + h0:k * S + h0 + 512],
                                start=(k == 0), stop=(k == ND - 1))
                        ht_dst = hTs[hf][mt // 8][:, (mt % 8) * 512:(mt % 8 + 1) * 512]
                        if mt % 2 == 0:
                            nc.scalar.activation(
                                ht_dst, ps[:],
                                Act.Relu, bias=b1_c[:, mt:mt + 1], scale=1.0)
                        else:
                            nc.vector.tensor_scalar(
                                ht_dst, ps[:],
                                b1_c[:, mt:mt + 1], 0.0, Alu.add, Alu.max)

                def stage_ffn2(hf):
                    h0 = hf * 512
                    for mt in range(ND):
                        ps = psbig.tile([128, 512], f32, tag="big")
                        for k in range(NH):
                            mmb(ps[:],
                                w2_s[:, k * D + mt * 128:k * D + (mt + 1) * 128],
                                hTs[hf][k // 8][:, (k % 8) * 512:(k % 8 + 1) * 512],
                                start=(k == 0), stop=(k == NH - 1))
                        nc.vector.scalar_tensor_tensor(
                            sum2T[:, mt * S + h0:mt * S + h0 + 512],
                            ps[:], b2_c[:, mt:mt + 1],
                            out1T_[:, mt * S + h0:mt * S + h0 + 512],
                            Alu.add, Alu.add)

                def stage_ln2(hf):
                    # x_out = LN2_out + seas + trend = LN2_out + x, so the
                    # seasonal residual folds into the final x_tok add
                    layernorm_T(psst, scr3, scr3s, sum2T, sum2T,
                                g2_c, be2_c, hf * 512)

                def stage_final(hf):
                    # transpose back + x add; LN3 stats via bn_stats; output
                    # DMAs spread across the three DMA-capable queues
                    xos = []
                    for i, st in enumerate(range(hf * 4, hf * 4 + 4)):
                        ps = psfin[0].tile([128, 512], bfdt, tag="bigtr")
                        for kt in range(ND):
                            nc.tensor.transpose(
                                ps[:, kt * 128:(kt + 1) * 128],
                                sum2T[:, kt * S + st * 128:kt * S + (st + 1) * 128],
                                identb)
                        xo_st = ph3.tile([128, D], f32, tag=f"xot{st % 4}")
                        xos.append(xo_st)
                        nc.vector.tensor_tensor(
                            xo_st[:], ps[:], x_tok[:, st * D:(st + 1) * D],
                            Alu.add)
                        bns = scr3.tile([128, 6], f32, tag="bns")
                        nc.vector.bn_stats(bns[:], xo_st[:])
                        nc.vector.bn_aggr(mv_all[:, 2 * st:2 * st + 2], bns[:])
                    hs = slice(hf * 4, hf * 4 + 4)
                    mv = mv_all[:].rearrange("p (s two) -> p s two", two=2)
                    means = mv[:, hs, 0]
                    varis = mv[:, hs, 1]
                    nc.vector.tensor_scalar(rstd3[:, hs], varis, EPS, None,
                                            Alu.add)
                    nc.scalar.activation(rstd3[:, hs], rstd3[:, hs],
                                         Act.Abs_reciprocal_sqrt,
                                         bias=0.0, scale=1.0)
                    nc.vector.scalar_tensor_tensor(
                        nb3[:, hs], means, -1.0, rstd3[:, hs],
                        Alu.mult, Alu.mult)
                    dmaq = [nc.sync, nc.scalar, nc.gpsimd, nc.sync]
                    for i, st in enumerate(range(hf * 4, hf * 4 + 4)):
                        xn = scr3.tile([128, D], f32, tag="xn")
                        nc.scalar.activation(
                            xn[:], xos[i][:],
                            Act.Identity, bias=nb3[:, st:st + 1],
                            scale=rstd3[:, st:st + 1])
                        nc.vector.tensor_tensor(xn[:], xn[:], g3bc[:],
                                                Alu.mult)
                        nc.gpsimd.tensor_tensor(xn[:], xn[:], be3bc[:],
                                                Alu.add)
                        dmaq[i].dma_start(out_d[st * 128:(st + 1) * 128, :],
                                          xn[:])

                hTs = {}
                psfin = {}
                stage_wo(0)
                stage_ln1(0)
                stage_wo(1)
                stage_ffn1(0)
                stage_ln1(1)
                stage_ffn2(0)
                stage_ffn1(1)
                stage_ln2(0)
                stage_ffn2(1)
                stage_ln2(1)
                psst_ctx.__exit__(None, None, None)
                psbig_ctx.__exit__(None, None, None)
                with tc.tile_pool(name="psfin", bufs=2,
                                  space="PSUM") as psfin_p:
                    psfin[0] = psfin_p
                    stage_final(0)
                    stage_final(1)

            wat_ctx.__exit__(None, None, None)
            w12_ctx.__exit__(None, None, None)

    nc.compile()
    return nc


def _get_nc():
    if "nc" not in _CACHE:
        _CACHE["nc"] = _build()
    return _CACHE["nc"]


def _pack_inputs(inputs):
    import ml_dtypes
    bf = ml_dtypes.bfloat16

    def packw(w):
        w = np.asarray(w, np.float32)
        din, dout = w.shape
        return (w.reshape(din // 128, 128, dout).transpose(1, 0, 2)
                .reshape(128, -1).astype(bf))

    # attention weights sum to 1 per row, so the v-bias passes through the
    # weighted average exactly: fold bv@wo into bo.
    bv = np.asarray(inputs["bv"], np.float64)
    wo = np.asarray(inputs["wo"], np.float64)
    bo = (np.asarray(inputs["bo"], np.float64) + bv @ wo).astype(np.float32)

    wqkvo = np.ascontiguousarray(np.concatenate(
        [packw(inputs["wq"]), packw(inputs["wk"]),
         packw(inputs["wv"]), packw(inputs["wo"])], axis=1))
    w12 = np.ascontiguousarray(np.concatenate(
        [packw(inputs["w1"]), packw(inputs["w2"])], axis=1))

    def colv(v):
        return np.asarray(v, np.float32).reshape(-1, 128).T

    colp = np.zeros((128, CO_W), np.float32)
    colp[:, CO_BQ:CO_BQ + ND] = colv(inputs["bq"])
    colp[:, CO_BK:CO_BK + ND] = colv(inputs["bk"])
    colp[:, CO_BO:CO_BO + ND] = colv(bo)
    colp[:, CO_B2:CO_B2 + ND] = colv(inputs["b2"])
    colp[:, CO_B1:CO_B1 + NH] = colv(inputs["b1"])
    colp[:, CO_G1:CO_G1 + ND] = colv(inputs["g1"])
    colp[:, CO_BE1:CO_BE1 + ND] = colv(inputs["be1"])
    colp[:, CO_G2:CO_G2 + ND] = colv(inputs["g2"])
    colp[:, CO_BE2:CO_BE2 + ND] = colv(inputs["be2"])
    g3be3 = np.concatenate(
        [np.asarray(inputs["g3"], np.float32),
         np.asarray(inputs["be3"], np.float32)]).reshape(1, 2 * D)
    return {
        "wqkvo": wqkvo,
        "w12": w12,
        "colpack": np.ascontiguousarray(colp),
        "g3be3": np.ascontiguousarray(g3be3),
    }


def kernel(**inputs):
    from concourse.bass_utils import run_bass_kernel_spmd

    nc = _get_nc()
    shared = _pack_inputs(inputs)
    x = np.ascontiguousarray(np.asarray(inputs["x"], np.float32))
    in_maps = [dict(shared, x=x[b]) for b in range(NCORES)]
    res = run_bass_kernel_spmd(nc, in_maps, list(range(NCORES)))
    out = np.stack([res.results[b]["out"] for b in range(NCORES)], axis=0)
    return out.astype(np.float32)


# revision 33
# speedup vs baseline: 1.1162x; 1.0560x over previous
"""Autoformer encoder block on 8 TRN2 NeuronCores.

Sharding: data-parallel over batch (B=8 -> 1 batch per core), weights
replicated. No collectives.

Per-core math (S=1024, D=512, H=8, dp=64, K=25):
  trend = movavg(x)               # banded matmul, token-major
  seas  = x - trend               # token-major, then PE-transpose -> seas.T
  q.T/k.T = wq/wk.T @ seas.T      # feature-major
  v     = seas @ wv               # token-major (for AV lhsT + V-sums)
  The reference's rfft/irfft over the depth axis (n=2S) makes
  corr[b,h,s,t] == 0 for t >= dp, so attention reduces to 64 depth-lags:
    corr.T = IDFT @ (QF (*) conj(KF)), QF = FWD.T @ q.T   (n=128 DFT)
    E = exp(corr/8); out = (E @ v[:64] + (Vsum - Vhead)) / (rowsum(E)+S-dp)
  wo, LN1, FFN(4x, relu), LN2 feature-major (stats via ones-matmul).
  seasonal_out + trend == x_out exactly (trend2 cancels), so movavg2 is
  skipped; final LN3 runs token-major after a PE-transpose, then DMA out.

Perf notes:
  - every heavy matmul is bf16 (weights cast+packed host-side into a few
    contiguous [128,N] blobs, activations bf16); residuals/stats f32
  - head is HBM-bound, so the critical path loads only ~1.6 MB (bf16 x
    + bf16 bands); weight blobs are gated behind those arrivals
  - DVE pressure is offloaded: PSUM operands are consumed in place
    (no staging copies), SBUF-only copies/reduces ride gpsimd, LN apply
    runs in bf16 with gains/biases folded into tensor_scalar
  - LN1/LN2 stats matmuls are interleaved into the producing wo/ffn2
    loops (one-iteration lag) so the LN finish is short
"""

import numpy as np

B, S, D, H = 8, 1024, 512, 8
DP = D // H
DH = 4 * D
KWIN, PAD = 25, 12
EPS = 1e-6
NCORES = 8
NT = S // 128   # 8 token tiles
ND = D // 128   # 4 feature tiles
NH = DH // 128  # 16 hidden tiles

_CACHE = {}

# CBANDS pack (bf16): bandm | bandpn | ident | ones
CA_BM, CA_BPN, CA_ID, CA_ONE = 0, 1024, 2048, 2176
CA_W = 2176 + 128
# CBF pack (bf16): fqd | fk2d | ia | ib | zblk | hsel
CB_FQD, CB_FK2, CB_IA, CB_IB, CB_ZBLK, CB_HSEL = 0, 128, 256, 320, 384, 416
CB_W = 416 + 512
# colpack layout: bq | bk | bo | b2 | b1 | g1 | be1 | g2 | be2
CO_BQ, CO_BK, CO_BO, CO_B2, CO_B1, CO_G1, CO_BE1, CO_G2, CO_BE2 = (
    0, 4, 8, 12, 16, 32, 36, 40, 44)
CO_W = 48


def _consts():
    import ml_dtypes
    bf = ml_dtypes.bfloat16
    c = {}
    # moving-average 3-piece band blocks over token-major x tiles:
    # trend tile j = BM[:,j]^T x[j] + BP[:,j]^T x[j-1][116:128] + BN[:,j]^T x[j+1][0:12]
    cnt = np.minimum(S, np.arange(S) + PAD + 1) - np.maximum(0, np.arange(S) - PAD)
    BM = np.zeros((128, NT * 128), np.float32)
    # BPN rows 0:12 = next-tile piece, rows 64:128 = prev-tile piece
    BPN = np.zeros((128, NT * 128), np.float32)
    for j in range(NT):
        for cc in range(128):
            s = 128 * j + cc
            for i in range(128):
                if abs(i - cc) <= PAD:
                    BM[i, 128 * j + cc] = 1.0 / cnt[s]
            for i in range(64, 128):
                if j >= 1 and abs(128 * (j - 1) + i - s) <= PAD:
                    BPN[i, 128 * j + cc] = 1.0 / cnt[s]
            for i in range(PAD):
                if j < NT - 1 and abs(128 * (j + 1) + i - s) <= PAD:
                    BPN[i, 128 * j + cc] = 1.0 / cnt[s]

    cbands = np.zeros((128, CA_W), np.float32)
    cbands[:, CA_BM:CA_BM + 1024] = BM
    cbands[:, CA_BPN:CA_BPN + 1024] = BPN
    cbands[:, CA_ID:CA_ID + 128] = np.eye(128)
    cbands[:, CA_ONE:CA_ONE + 128] = 1.0
    c["CBANDS"] = cbands.astype(bf)

    # packed forward DFT (n=128): FQ [64,128] = [cos f=0..64 | sin f=1..63],
    # FK2 [64,128] = [sin f=0..64 | cos f=1..63]; doubled over partitions so
    # heads at base 0 and base 64 can slice the same constant.
    n = 2 * DP
    d = np.arange(DP)[:, None]
    f65 = np.arange(65)[None, :]
    f63 = np.arange(1, 64)[None, :]
    FQ = np.concatenate([np.cos(2 * np.pi * f65 * d / n),
                         np.sin(2 * np.pi * f63 * d / n)], axis=1)
    FK2 = np.concatenate([np.sin(2 * np.pi * f65 * d / n),
                          np.cos(2 * np.pi * f63 * d / n)], axis=1)

    # packed inverse (softmax scale 1/sqrt(dp) folded in):
    # corr'[t] = IA^T @ (qf*k1f) + IB^T @ (qf*k2f)
    t = np.arange(DP)[None, :]
    w = np.full(65, 2.0); w[0] = 1.0; w[64] = 1.0
    fc = np.arange(65)[:, None]
    fs = np.arange(1, 64)[:, None]
    scale = 1.0 / np.sqrt(DP)
    IA = np.concatenate([(w[:, None] / n) * np.cos(2 * np.pi * fc * t / n),
                         (2.0 / n) * np.cos(2 * np.pi * fs * t / n)], axis=0)
    IB = np.concatenate([-(w[:, None] / n) * np.sin(2 * np.pi * fc * t / n),
                         (2.0 / n) * np.sin(2 * np.pi * fs * t / n)], axis=0)

    # Z rows: ZBLK[:, 8p+j] sums E-pair rows 0:64 into row 2p, 64:128 into 2p+1
    ZBLK = np.zeros((128, 32), np.float32)
    for p in range(4):
        ZBLK[0:64, 8 * p + 2 * p] = 1.0
        ZBLK[64:128, 8 * p + 2 * p + 1] = 1.0

    # head-pair selector for 1/Z broadcast: pass p covers heads 2p, 2p+1
    HSEL = np.zeros((128, 4 * 128), np.float32)
    for p in range(4):
        for mm_ in range(128):
            HSEL[2 * p + mm_ // 64, p * 128 + mm_] = 1.0

    cbf = np.zeros((128, CB_W), np.float32)
    cbf[:, CB_FQD:CB_FQD + 128] = np.concatenate([FQ, FQ], axis=0)
    cbf[:, CB_FK2:CB_FK2 + 128] = np.concatenate([FK2, FK2], axis=0)
    cbf[:, CB_IA:CB_IA + DP] = scale * IA
    cbf[:, CB_IB:CB_IB + DP] = scale * IB
    cbf[:, CB_ZBLK:CB_ZBLK + 32] = ZBLK
    cbf[:, CB_HSEL:CB_HSEL + 512] = HSEL
    c["CBF"] = cbf.astype(bf)
    return c


def _build():
    import concourse.bacc as bacc
    import concourse.mybir as mybir
    import concourse.tile as tile

    f32 = mybir.dt.float32
    f32r = mybir.dt.float32r
    bfdt = mybir.dt.bfloat16
    Alu = mybir.AluOpType
    Act = mybir.ActivationFunctionType

    nc = bacc.Bacc()

    # ---- DRAM parameters (host-packed) ----
    xbf_d = nc.declare_dram_parameter("xbf", [128, NT * D], bfdt,
                                      isOutput=False)
    wqkvo_d = nc.declare_dram_parameter("wqkvo", [128, 4 * ND * D], bfdt,
                                        isOutput=False)
    w12_d = nc.declare_dram_parameter("w12", [128, 2 * ND * DH], bfdt,
                                      isOutput=False)
    colp_d = nc.declare_dram_parameter("colpack", [128, CO_W], f32,
                                       isOutput=False)
    g3be3_d = nc.declare_dram_parameter("g3be3", [1, 2 * D], bfdt,
                                        isOutput=False)
    out_d = nc.declare_dram_parameter("out", [S, D], f32, isOutput=True)

    C = _consts()
    CBANDS_d = nc.inline_tensor(C["CBANDS"], name="c_bands")
    CBF_d = nc.inline_tensor(C["CBF"], name="c_bf")

    def r(ap):
        return ap.bitcast(f32r)

    def mm(out, lhsT, rhs, start=True, stop=True):
        nc.tensor.matmul(out, r(lhsT), r(rhs), start=start, stop=stop)

    def mmb(out, lhsT, rhs, start=True, stop=True):
        nc.tensor.matmul(out, lhsT, rhs, start=start, stop=stop)

    with tile.TileContext(nc) as tc:
        with (
            tc.tile_pool(name="konst", bufs=1) as konst,
            tc.tile_pool(name="persist", bufs=1) as persist,
            tc.tile_pool(name="mid", bufs=1) as mid,
        ):
            # ---- packed consts to SBUF (few triggers, critical first) ----
            cbands = konst.tile([128, CA_W], bfdt)
            cbf = konst.tile([128, CB_W], bfdt)
            colp = konst.tile([128, CO_W], f32)
            g3be3_r = konst.tile([1, 2 * D], bfdt)
            x_bf = persist.tile([128, NT * D], bfdt)
            seasT = persist.tile([128, ND * S], bfdt)

            # critical path: bf16 x + bands only (~1.6 MB)
            nc.sync.dma_start(x_bf[:], xbf_d[:])
            nc.sync.dma_start(cbands[:], CBANDS_d[:])
            nc.scalar.dma_start(cbf[:], CBF_d[:])
            nc.scalar.dma_start(colp[:], colp_d[:])
            nc.scalar.dma_start(g3be3_r[:], g3be3_d[:])

            bandm = cbands[:, CA_BM:CA_BM + 1024]
            bandpn = cbands[:, CA_BPN:CA_BPN + 1024]
            identb = cbands[:, CA_ID:CA_ID + 128]
            onesb = cbands[:, CA_ONE:CA_ONE + 128]
            fqd = cbf[:, CB_FQD:CB_FQD + 128]
            fk2d = cbf[:, CB_FK2:CB_FK2 + 128]
            ia_b = cbf[:, CB_IA:CB_IA + DP]
            ib_b = cbf[:, CB_IB:CB_IB + DP]
            zblk = cbf[:, CB_ZBLK:CB_ZBLK + 32]
            hsel = cbf[0:8, CB_HSEL:CB_HSEL + 512]
            bq_c = colp[:, CO_BQ:CO_BQ + ND]
            bk_c = colp[:, CO_BK:CO_BK + ND]
            bo_c = colp[:, CO_BO:CO_BO + ND]
            b2_c = colp[:, CO_B2:CO_B2 + ND]
            b1_c = colp[:, CO_B1:CO_B1 + NH]
            g1_c = colp[:, CO_G1:CO_G1 + ND]
            be1_c = colp[:, CO_BE1:CO_BE1 + ND]
            g2_c = colp[:, CO_G2:CO_G2 + ND]
            be2_c = colp[:, CO_BE2:CO_BE2 + ND]
            g3_r = g3be3_r[0:1, 0:D]
            be3_r = g3be3_r[0:1, D:2 * D]

            # weight blobs: wqkvo gated on x arrival (scalar queue), w12
            # gated on movavg progress (gpsimd queue) — keeps the head's
            # HBM bandwidth for the critical x/bands loads
            w12_ctx = tc.tile_pool(name="w12p", bufs=1)
            w12p = w12_ctx.__enter__()
            wat_ctx = tc.tile_pool(name="wat", bufs=1)
            wat = wat_ctx.__enter__()
            wqkvo_s = wat.tile([128, 4 * ND * D], bfdt)
            gate1 = konst.tile([1, 1], bfdt)
            nc.scalar.dma_start(gate1[:], x_bf[0:1, 0:1])
            nc.scalar.dma_start(wqkvo_s[:], wqkvo_d[:])
            wq_s = wqkvo_s[:, 0 * ND * D:1 * ND * D]
            wk_s = wqkvo_s[:, 1 * ND * D:2 * ND * D]
            wv_s = wqkvo_s[:, 2 * ND * D:3 * ND * D]
            wo_s = wqkvo_s[:, 3 * ND * D:4 * ND * D]
            w12_s = w12p.tile([128, 2 * ND * DH], bfdt)
            w1_s = w12_s[:, 0:ND * DH]
            w2_s = w12_s[:, ND * DH:2 * ND * DH]

            # ============ phase 1: moving average, transpose ========
            with (
                tc.tile_pool(name="ph1", bufs=1) as ph1,
                tc.tile_pool(name="ps1", bufs=2, space="PSUM") as ps1,
            ):
                # PE warm-up during the head DMA wait: ramps the clock so
                # the movavg/transpose chain starts at full speed
                warm = ps1.tile([128, 128], bfdt, tag="warm")
                for _ in range(20):
                    nc.tensor.transpose(warm[:], identb, identb)

                seas_tok = ph1.tile([128, NT * D], bfdt)
                for j in range(NT):
                    ps = ps1.tile([128, D], f32, tag="mavg")
                    mmb(ps[:], bandm[:, 128 * j:128 * (j + 1)],
                        x_bf[:, j * D:(j + 1) * D], start=True, stop=False)
                    if j >= 1:
                        mmb(ps[:], bandpn[64:128, 128 * j:128 * (j + 1)],
                            x_bf[64:128, (j - 1) * D:j * D],
                            start=False, stop=(j == NT - 1))
                    if j < NT - 1:
                        mmb(ps[:], bandpn[0:PAD, 128 * j:128 * (j + 1)],
                            x_bf[0:PAD, (j + 1) * D:(j + 2) * D],
                            start=False, stop=True)
                    nc.vector.tensor_tensor(
                        seas_tok[:, j * D:(j + 1) * D],
                        x_bf[:, j * D:(j + 1) * D], ps[:], Alu.subtract)

                # w12 load gated on movavg progress (gpsimd DMA queue)
                gate = ph1.tile([1, 1], bfdt)
                nc.gpsimd.dma_start(gate[:], seas_tok[0:1, 3 * D:3 * D + 1])
                nc.gpsimd.dma_start(w12_s[:], w12_d[:])

                for st in range(NT):
                    for kt in range(ND):
                        ps = ps1.tile([128, 128], bfdt, tag="tr")
                        nc.tensor.transpose(
                            ps[:],
                            seas_tok[:, st * D + kt * 128:st * D + (kt + 1) * 128],
                            identb)
                        dst = seasT[:, kt * S + st * 128:kt * S + (st + 1) * 128]
                        if (st + kt) % 2 == 0:
                            nc.vector.tensor_copy(dst, ps[:])
                        else:
                            nc.scalar.activation(dst, ps[:], Act.Identity,
                                                 bias=0.0, scale=1.0)

            # ============ phase 2: attention ============
            out1T = mid.tile([128, ND * S], bfdt, tag="m16")
            with (
                tc.tile_pool(name="ph2", bufs=1) as ph2,
                tc.tile_pool(name="att1", bufs=1) as att1,
                tc.tile_pool(name="scr2", bufs=2) as scr2,
            ):
                qT = ph2.tile([128, ND * S], bfdt)
                kT = ph2.tile([128, ND * S], bfdt)
                attnT = mid.tile([128, ND * S], bfdt, tag="attn")
                vb2 = att1.tile([128, H * DP], bfdt)

                with tc.tile_pool(name="psqkv", bufs=2, space="PSUM") as psq:
                    for w_s, bcol, dst in ((wq_s, bq_c, qT), (wk_s, bk_c, kT)):
                        for mt in range(ND):
                            ps = psq.tile([128, S], f32, tag="big")
                            for nn in range(2):
                                for k in range(ND):
                                    mmb(ps[:, nn * 512:(nn + 1) * 512],
                                        w_s[:, k * D + mt * 128:k * D + (mt + 1) * 128],
                                        seasT[:, k * S + nn * 512:k * S + (nn + 1) * 512],
                                        start=(k == 0), stop=(k == ND - 1))
                            nc.scalar.activation(
                                dst[:, mt * S:(mt + 1) * S], ps[:],
                                Act.Identity, bias=bcol[:, mt:mt + 1], scale=1.0)
                    # v at lags 0..63 only, stacked twice over partitions so
                    # both halves of a head-pair block-diagonal lhsT exist
                    vsrc = att1.tile([128, ND * 128], bfdt)
                    for k in range(ND):
                        for half in range(2):
                            nc.gpsimd.tensor_copy(
                                vsrc[:, k * 128 + 64 * half:k * 128 + 64 * half + 64],
                                seasT[:, k * S:k * S + DP])
                    ps = psq.tile([128, D], f32, tag="vtok")
                    for k in range(ND):
                        mmb(ps[:], vsrc[:, k * 128:(k + 1) * 128],
                            wv_s[:, k * D:(k + 1) * D],
                            start=(k == 0), stop=(k == ND - 1))
                    nc.scalar.activation(vb2[:], ps[:], Act.Identity,
                                         bias=0.0, scale=1.0)
                # block-diagonal [v_even | v_odd] lhsT per pair (gpsimd —
                # SBUF-only traffic, keeps DVE free for the DFT products)
                vblk = att1.tile([128, 4 * 128], bfdt)
                nc.gpsimd.memset(vblk[:], 0.0)
                for p in range(4):
                    nc.gpsimd.tensor_copy(
                        vblk[0:64, p * 128:p * 128 + 64],
                        vb2[0:64, 128 * p:128 * p + 64])
                    nc.gpsimd.tensor_copy(
                        vblk[64:128, p * 128 + 64:(p + 1) * 128],
                        vb2[64:128, 128 * p + 64:128 * p + 128])
                # lag-sums for W = Vsum - Vhead (gpsimd, SBUF-only)
                sdif = att1.tile([128, 2 * ND], bfdt)
                for k in range(ND):
                    with nc.allow_low_precision(reason="bf16 lag-sum is ample"):
                        nc.vector.tensor_reduce(
                            sdif[:, 2 * k:2 * k + 1],
                            seasT[:, k * S + DP:(k + 1) * S],
                            mybir.AxisListType.X, Alu.add)
                    nc.gpsimd.tensor_copy(sdif[:, 2 * k + 1:2 * k + 2],
                                          sdif[:, 2 * k:2 * k + 1])

                # E_all pair-major: block p = heads (2p, 2p+1) at rows (0:64,
                # 64:128), cols = tokens
                E_all = att1.tile([128, 4 * S], bfdt)

                # phase A: packed DFT -> 2 products -> packed inverse -> E
                with (
                    tc.tile_pool(name="psF", bufs=2, space="PSUM") as psF,
                    tc.tile_pool(name="psC", bufs=2, space="PSUM") as psC,
                ):
                    for p in range(4):
                        for nn in range(2):
                            c0 = p * S + nn * 512
                            cr = psC.tile([128, 512], f32, tag="corr")
                            for e in range(2):
                                ro = 64 * e
                                fq_l = fqd[ro:ro + 64, :]
                                fk2_l = fk2d[ro:ro + 64, :]
                                qh = qT[ro:ro + 64, c0:c0 + 512]
                                kh = kT[ro:ro + 64, c0:c0 + 512]
                                qf = psF.tile([128, 512], f32, tag="qf")
                                k1 = psF.tile([128, 512], f32, tag="k1")
                                k2 = psF.tile([128, 512], f32, tag="k2")
                                nc.tensor.matmul(qf[:], fq_l, qh,
                                                 start=True, stop=True)
                                nc.tensor.matmul(k1[:], fq_l, kh,
                                                 start=True, stop=True)
                                nc.tensor.matmul(k2[:], fk2_l, kh,
                                                 start=True, stop=True)
                                qf_sb = scr2.tile([128, 512], bfdt,
                                                  tag="qfsb")
                                nc.vector.tensor_copy(qf_sb[:], qf[:])
                                ab = scr2.tile([128, S], bfdt, tag="ab")
                                nc.vector.tensor_tensor(ab[:, 0:512],
                                                        qf_sb[:],
                                                        k1[:], Alu.mult)
                                nc.vector.tensor_tensor(ab[:, 512:1024],
                                                        qf_sb[:], k2[:],
                                                        Alu.mult)
                                nc.tensor.matmul(cr[ro:ro + 64, :], ia_b[:],
                                                 ab[:, 0:512],
                                                 start=True, stop=False)
                                nc.tensor.matmul(cr[ro:ro + 64, :], ib_b[:],
                                                 ab[:, 512:1024],
                                                 start=False, stop=True)
                            nc.scalar.activation(E_all[:, c0:c0 + 512],
                                                 cr[:], Act.Exp, bias=0.0,
                                                 scale=1.0)

                # W = Vsum - Vhead = (sum_{s>=64} seas.T[:,s]) @ wv
                # (emitted after the DFT so these PE ops fill the 1/Z
                # latency instead of stalling the DFT pipeline)
                W_sb = att1.tile([128, ND], f32)
                zsum = att1.tile([8, S], f32)
                zinv = att1.tile([8, S], f32)
                zinv_b = att1.tile([8, S], bfdt)
                with (
                    tc.tile_pool(name="psZ", bufs=1, space="PSUM") as psZ,
                    tc.tile_pool(name="psw", bufs=1, space="PSUM") as psw,
                ):
                    Zall = psZ.tile([8, S], f32, tag="zall")
                    for nn in range(2):
                        sl = slice(nn * 512, (nn + 1) * 512)
                        for p in range(4):
                            nc.tensor.matmul(
                                Zall[:, sl], zblk[:, 8 * p:8 * p + 8],
                                E_all[:, p * S + nn * 512:p * S + (nn + 1) * 512],
                                start=(p == 0), stop=(p == 3))
                    nc.vector.tensor_scalar(zsum[:], Zall[:], float(S - DP),
                                            None, Alu.add)
                    ps_w2 = psw.tile([128, 2 * ND], f32, tag="w2")
                    for mt in range(ND):
                        for k in range(ND):
                            mmb(ps_w2[:, 2 * mt:2 * mt + 2],
                                wv_s[:, k * D + mt * 128:k * D + (mt + 1) * 128],
                                sdif[:, 2 * k:2 * k + 2],
                                start=(k == 0), stop=(k == ND - 1))
                    with nc.allow_low_precision(reason="1/Z approx is ample"):
                        nc.vector.reciprocal_approx_fast(out=zinv[:],
                                                         in_=zsum[:])
                        nc.vector.tensor_copy(zinv_b[:], zinv[:])
                    nc.vector.tensor_copy(
                        W_sb[:],
                        ps_w2[:].rearrange("p (a b) -> p a b", b=2)[:, :, 0])

                with (
                    tc.tile_pool(name="psB2", bufs=2, space="PSUM") as psB2,
                    tc.tile_pool(name="psZb", bufs=2, space="PSUM") as psZb,
                ):
                    for nn in range(2):
                        for p in range(4):
                            sl = slice(nn * 512, (nn + 1) * 512)
                            nv = psB2.tile([128, 512], f32, tag="nv")
                            nc.tensor.matmul(
                                nv[:], vblk[:, p * 128:(p + 1) * 128],
                                E_all[:, p * S + nn * 512:p * S + (nn + 1) * 512],
                                start=True, stop=True)
                            zbc = psZb.tile([128, 512], f32, tag="zbc")
                            mmb(zbc[:], hsel[:, p * 128:(p + 1) * 128],
                                zinv_b[:, sl])
                            zbc_sb = scr2.tile([128, 512], bfdt, tag="zbcsb")
                            nc.scalar.activation(zbc_sb[:], zbc[:],
                                                 Act.Identity,
                                                 bias=0.0, scale=1.0)
                            nc.vector.scalar_tensor_tensor(
                                attnT[:, p * S + nn * 512:p * S + (nn + 1) * 512],
                                nv[:], W_sb[:, p:p + 1], zbc_sb[:],
                                Alu.add, Alu.mult)

            out1T_ = out1T

            # ============ back end ============
            with (
                tc.tile_pool(name="ph3", bufs=1) as ph3,
                tc.tile_pool(name="scr3", bufs=2) as scr3,
                tc.tile_pool(name="scr3s", bufs=1) as scr3s,
                tc.tile_pool(name="hTp", bufs=1) as hTp,
            ):
                psbig_ctx = tc.tile_pool(name="psbig", bufs=3, space="PSUM")
                psbig = psbig_ctx.__enter__()
                psst_ctx = tc.tile_pool(name="psst", bufs=2, space="PSUM")
                psst = psst_ctx.__enter__()
                sum2T = attnT
                g3p = psbig.tile([128, D], f32, tag="big")
                be3p = psbig.tile([128, D], f32, tag="big")
                mmb(g3p[:], onesb[0:1, :], g3_r)
                mmb(be3p[:], onesb[0:1, :], be3_r)
                g3bc = scr3s.tile([128, D], f32, tag="g3bc")
                be3bc = scr3s.tile([128, D], f32, tag="be3bc")
                nc.vector.tensor_copy(g3bc[:], g3p[:])
                nc.vector.tensor_copy(be3bc[:], be3p[:])
                mv_all = scr3s.tile([128, 2 * NT], f32, tag="st3mv")
                rstd3 = scr3s.tile([128, NT], f32, tag="st3r")
                nb3 = scr3s.tile([128, NT], f32, tag="st3nb")

                # LN stats matmuls ride inside the producing loops with a
                # one-iteration lag (so the PE never waits on the DVE fuse)
                def emit_stats(stt, src, mt, h0):
                    c0 = mt * S + h0
                    sq = scr3.tile([128, 512], bfdt, tag="lnsq")
                    sq_eng = nc.gpsimd if mt % 2 == 0 else nc.vector
                    sq_eng.tensor_tensor(sq[:], src[:, c0:c0 + 512],
                                         src[:, c0:c0 + 512], Alu.mult)
                    mmb(stt[0:1, :], onesb[:, 0:1], src[:, c0:c0 + 512],
                        start=(mt == 0), stop=(mt == ND - 1))
                    mmb(stt[32:33, :], onesb[:, 1:2], sq[:],
                        start=(mt == 0), stop=(mt == ND - 1))

                def ln_finish(stt, src, dst, g_c, be_c, h0):
                    s1 = stt[0:1, :]
                    s2 = stt[32:33, :]
                    mean = scr3s.tile([1, 512], f32, tag="lnm")
                    msq = scr3s.tile([1, 512], f32, tag="lnq")
                    var = scr3s.tile([1, 512], f32, tag="lnv")
                    rstd = scr3s.tile([1, 512], bfdt, tag="lnr")
                    bbn = scr3s.tile([1, 512], f32, tag="lnv")
                    bb = scr3s.tile([1, 512], bfdt, tag="lnm")
                    nc.vector.tensor_scalar(mean[:], s1, 1.0 / D, None,
                                            Alu.mult)
                    nc.vector.tensor_scalar(var[:], s2, 1.0 / D, EPS,
                                            Alu.mult, Alu.add)
                    nc.vector.tensor_tensor(msq[:], mean[:], mean[:], Alu.mult)
                    nc.vector.tensor_tensor(var[:], var[:], msq[:],
                                            Alu.subtract)
                    nc.scalar.activation(rstd[:], var[:],
                                         Act.Abs_reciprocal_sqrt,
                                         bias=0.0, scale=1.0)
                    nc.vector.tensor_scalar(bbn[:], mean[:], -1.0, None,
                                            Alu.mult)
                    nc.vector.tensor_tensor(bb[:], bbn[:], rstd[:], Alu.mult)
                    ab2 = psst.tile([128, 1024], f32, tag="lnAB", bufs=1)
                    mmb(ab2[:, 0:512], onesb[0:1, :], rstd[:])
                    mmb(ab2[:, 512:1024], onesb[0:1, :], bb[:])
                    absb = scr3.tile([128, 1024], bfdt, tag="lnab")
                    nc.vector.tensor_copy(absb[:], ab2[:])
                    for k in range(ND):
                        c0 = k * S + h0
                        eng = nc.gpsimd if k % 2 == 0 else nc.vector
                        t = scr3.tile([128, 512], bfdt, tag="lnt")
                        eng.tensor_tensor(t[:], src[:, c0:c0 + 512],
                                          absb[:, 0:512], Alu.mult)
                        eng.tensor_tensor(t[:], t[:], absb[:, 512:1024],
                                          Alu.add)
                        eng.tensor_scalar(dst[:, c0:c0 + 512], t[:],
                                          g_c[:, k:k + 1], be_c[:, k:k + 1],
                                          Alu.mult, Alu.add)

                def stage_wo(hf):
                    h0 = hf * 512
                    stt = psst.tile([33, 512], f32, tag="stat")
                    stts["ln1", hf] = stt
                    for mt in range(ND):
                        ps = psbig.tile([128, 512], f32, tag="big")
                        for k in range(ND):
                            mmb(ps[:],
                                wo_s[:, k * D + mt * 128:k * D + (mt + 1) * 128],
                                attnT[:, k * S + h0:k * S + h0 + 512],
                                start=(k == 0), stop=(k == ND - 1))
                        nc.vector.scalar_tensor_tensor(
                            out1T_[:, mt * S + h0:mt * S + h0 + 512],
                            ps[:], bo_c[:, mt:mt + 1],
                            seasT[:, mt * S + h0:mt * S + h0 + 512],
                            Alu.add, Alu.add)
                        if mt > 0:
                            emit_stats(stt, out1T_, mt - 1, h0)
                    emit_stats(stt, out1T_, ND - 1, h0)

                def stage_ln1(hf):
                    ln_finish(stts["ln1", hf], out1T_, out1T_,
                              g1_c, be1_c, hf * 512)

                def stage_ffn1(hf):
                    h0 = hf * 512
                    hTa = hTp.tile([128, 8 * 512], bfdt, tag="hTa")
                    hTb = hTp.tile([128, 8 * 512], bfdt, tag="hTb")
                    hTs[hf] = [hTa, hTb]
                    for mt in range(NH):
                        ps = psbig.tile([128, 512], f32, tag="big")
                        for k in range(ND):
                            mmb(ps[:],
                                w1_s[:, k * DH + mt * 128:k * DH + (mt + 1) * 128],
                                out1T_[:, k * S + h0:k * S + h0 + 512],
                                start=(k == 0), stop=(k == ND - 1))
                        ht_dst = hTs[hf][mt // 8][:, (mt % 8) * 512:(mt % 8 + 1) * 512]
                        if mt % 2 == 0:
                            nc.scalar.activation(
                                ht_dst, ps[:],
                                Act.Relu, bias=b1_c[:, mt:mt + 1], scale=1.0)
                        else:
                            nc.vector.tensor_scalar(
                                ht_dst, ps[:],
                                b1_c[:, mt:mt + 1], 0.0, Alu.add, Alu.max)

                def stage_ffn2(hf):
                    h0 = hf * 512
                    stt = psst.tile([33, 512], f32, tag="stat")
                    stts["ln2", hf] = stt
                    for mt in range(ND):
                        ps = psbig.tile([128, 512], f32, tag="big")
                        for k in range(NH):
                            mmb(ps[:],
                                w2_s[:, k * D + mt * 128:k * D + (mt + 1) * 128],
                                hTs[hf][k // 8][:, (k % 8) * 512:(k % 8 + 1) * 512],
                                start=(k == 0), stop=(k == NH - 1))
                        nc.vector.scalar_tensor_tensor(
                            sum2T[:, mt * S + h0:mt * S + h0 + 512],
                            ps[:], b2_c[:, mt:mt + 1],
                            out1T_[:, mt * S + h0:mt * S + h0 + 512],
                            Alu.add, Alu.add)
                        if mt > 0:
                            emit_stats(stt, sum2T, mt - 1, h0)
                    emit_stats(stt, sum2T, ND - 1, h0)

                def stage_ln2(hf):
                    # x_out = LN2_out + seas + trend = LN2_out + x, so the
                    # seasonal residual folds into the final x_tok add
                    ln_finish(stts["ln2", hf], sum2T, sum2T,
                              g2_c, be2_c, hf * 512)

                def stage_final(hf):
                    # kt-major transposes (each starts as soon as the ln2
                    # apply for that feature tile lands), then per-tile
                    # normalize; output DMAs rotate across the three queues
                    h0 = hf * 512
                    pss = [psfin[0].tile([128, 512], bfdt, tag=f"tr{i}",
                                         name=f"trp{i}")
                           for i in range(4)]
                    for kt in range(ND):
                        for i, st in enumerate(range(hf * 4, hf * 4 + 4)):
                            nc.tensor.transpose(
                                pss[i][:, kt * 128:(kt + 1) * 128],
                                sum2T[:, kt * S + st * 128:kt * S + (st + 1) * 128],
                                identb)
                    xos = []
                    for i, st in enumerate(range(hf * 4, hf * 4 + 4)):
                        xo_st = ph3.tile([128, D], f32, tag=f"xot{st % 4}")
                        xos.append(xo_st)
                        nc.vector.tensor_tensor(
                            xo_st[:], pss[i][:], x_bf[:, st * D:(st + 1) * D],
                            Alu.add)
                        bns = scr3.tile([128, 6], f32, tag="bns")
                        nc.vector.bn_stats(bns[:], xo_st[:])
                        nc.vector.bn_aggr(mv_all[:, 2 * st:2 * st + 2], bns[:])
                    hs = slice(hf * 4, hf * 4 + 4)
                    mv = mv_all[:].rearrange("p (s two) -> p s two", two=2)
                    means = mv[:, hs, 0]
                    varis = mv[:, hs, 1]
                    nc.vector.tensor_scalar(rstd3[:, hs], varis, EPS, None,
                                            Alu.add)
                    nc.scalar.activation(rstd3[:, hs], rstd3[:, hs],
                                         Act.Abs_reciprocal_sqrt,
                                         bias=0.0, scale=1.0)
                    nc.vector.scalar_tensor_tensor(
                        nb3[:, hs], means, -1.0, rstd3[:, hs],
                        Alu.mult, Alu.mult)
                    dmaq = [nc.sync, nc.scalar, nc.gpsimd, nc.sync]
                    for i, st in enumerate(range(hf * 4, hf * 4 + 4)):
                        eng = nc.gpsimd if i % 2 == 0 else nc.vector
                        y1 = scr3.tile([128, D], bfdt, tag="y1")
                        xn = scr3.tile([128, D], f32, tag="xn")
                        eng.tensor_scalar(y1[:], xos[i][:],
                                          rstd3[:, st:st + 1],
                                          nb3[:, st:st + 1],
                                          Alu.mult, Alu.add)
                        eng.tensor_tensor(y1[:], y1[:], g3bc[:], Alu.mult)
                        eng.tensor_tensor(xn[:], y1[:], be3bc[:], Alu.add)
                        dmaq[i].dma_start(out_d[st * 128:(st + 1) * 128, :],
                                          xn[:])

                hTs = {}
                stts = {}
                psfin = {}
                stage_wo(0)
                stage_ln1(0)
                stage_wo(1)
                stage_ffn1(0)
                stage_ln1(1)
                stage_ffn2(0)
                stage_ffn1(1)
                stage_ln2(0)
                stage_ffn2(1)
                stage_ln2(1)
                psst_ctx.__exit__(None, None, None)
                psbig_ctx.__exit__(None, None, None)
                with tc.tile_pool(name="psfin", bufs=1,
                                  space="PSUM") as psfin_p:
                    psfin[0] = psfin_p
                    stage_final(0)
                    stage_final(1)

            wat_ctx.__exit__(None, None, None)
            w12_ctx.__exit__(None, None, None)

    nc.compile()
    return nc


def _get_nc():
    if "nc" not in _CACHE:
        _CACHE["nc"] = _build()
    return _CACHE["nc"]


def _pack_inputs(inputs):
    import ml_dtypes
    bf = ml_dtypes.bfloat16

    def packw(w):
        w = np.asarray(w, np.float32)
        din, dout = w.shape
        return (w.reshape(din // 128, 128, dout).transpose(1, 0, 2)
                .reshape(128, -1).astype(bf))

    # attention weights sum to 1 per row, so the v-bias passes through the
    # weighted average exactly: fold bv@wo into bo.
    bv = np.asarray(inputs["bv"], np.float64)
    wo = np.asarray(inputs["wo"], np.float64)
    bo = (np.asarray(inputs["bo"], np.float64) + bv @ wo).astype(np.float32)

    wqkvo = np.ascontiguousarray(np.concatenate(
        [packw(inputs["wq"]), packw(inputs["wk"]),
         packw(inputs["wv"]), packw(inputs["wo"])], axis=1))
    w12 = np.ascontiguousarray(np.concatenate(
        [packw(inputs["w1"]), packw(inputs["w2"])], axis=1))

    def colv(v):
        return np.asarray(v, np.float32).reshape(-1, 128).T

    colp = np.zeros((128, CO_W), np.float32)
    colp[:, CO_BQ:CO_BQ + ND] = colv(inputs["bq"])
    colp[:, CO_BK:CO_BK + ND] = colv(inputs["bk"])
    colp[:, CO_BO:CO_BO + ND] = colv(bo)
    colp[:, CO_B2:CO_B2 + ND] = colv(inputs["b2"])
    colp[:, CO_B1:CO_B1 + NH] = colv(inputs["b1"])
    colp[:, CO_G1:CO_G1 + ND] = colv(inputs["g1"])
    colp[:, CO_BE1:CO_BE1 + ND] = colv(inputs["be1"])
    colp[:, CO_G2:CO_G2 + ND] = colv(inputs["g2"])
    colp[:, CO_BE2:CO_BE2 + ND] = colv(inputs["be2"])
    g3be3 = np.concatenate(
        [np.asarray(inputs["g3"], np.float32),
         np.asarray(inputs["be3"], np.float32)]).reshape(1, 2 * D).astype(bf)
    return {
        "wqkvo": wqkvo,
        "w12": w12,
        "colpack": np.ascontiguousarray(colp),
        "g3be3": np.ascontiguousarray(g3be3),
    }


def _pack_x(xb):
    import ml_dtypes
    bf = ml_dtypes.bfloat16
    # token-major SBUF layout: [128, (st d)] with partition = token % 128
    return np.ascontiguousarray(
        np.asarray(xb, np.float32).reshape(NT, 128, D).transpose(1, 0, 2)
        .reshape(128, NT * D).astype(bf))


def kernel(**inputs):
    from concourse.bass_utils import run_bass_kernel_spmd

    nc = _get_nc()
    shared = _pack_inputs(inputs)
    x = np.asarray(inputs["x"], np.float32)
    in_maps = [dict(shared, xbf=_pack_x(x[b])) for b in range(NCORES)]
    res = run_bass_kernel_spmd(nc, in_maps, list(range(NCORES)))
    out = np.stack([res.results[b]["out"] for b in range(NCORES)], axis=0)
    return out.astype(np.float32)
